# revision 1
# baseline (speedup 1.0000x reference)
"""Causal multi-head attention (B=8, T=1024, E=768, H=12, D=64) on 8 trn2
NeuronCores, data-parallel over the batch (one batch element per core).

v3 — f32r matmul datapath (bf16 emits LDWEIGHTS-per-matmul which serializes
on this walrus build) with fine-grained projection/attention interleaving.

Per-core pipeline (all matmuls float32r — full PE rate, ~1e-4 rel err):
  1. PE warmup matmuls ride out the cold-clock window while x/W DMAs stream
     (first-needed-first DMA order across the sync + scalar DGE queues).
  2. Q^T = (Wq/8) @ x^T + bq/8, K^T = Wk @ x^T + bk -> SBUF f32r [768, 1024]
     (softmax 1/sqrt(D) scale folded into Wq/bq on the host).
  3. V = x @ Wv^T (bias folded into the output projection bias) -> SBUF
     [1024, 12*65] with a ones column per head ("V65") so the attention
     context matmul also produces the softmax denominator.
  4. Per head pair, per 512-wide q window: S^T[k,q] via row-packed matmuls
     (K=64, tile_position (0,0)/(64,0)); the 128-wide triangle-edge tiles
     are widened to 256 into the never-read masked zone to dodge the f32r
     sub-256 4-cycle/row penalty; causal mask added on both heads'
     diagonal 128-blocks by one strided DVE op (free-dim stride-0 mask
     broadcast), exp on ACT -> f32r P^T, ctx^T[65,q] accumulated over
     k-chunks with V65 stationary.  Projection matmuls for the NEXT head
     pair are emitted between the k-chunk steps as PE filler so exp latency
     never stalls the in-order PE queue.
  5. Normalization: ctx is copied PSUM->SBUF on ACT (its window-boundary
     lull) right after the last PV so the PSUM accumulators free for the
     next window without queuing behind DVE; then (off the critical path) reciprocal of row 64 on DVE (partitions 0/32), broadcast across
     partitions via a DRAM-bounce DMA (stride-0 partition source; a PE
     rank-2 broadcast matmul measures ~32us on this HW), and GPSIMD
     multiplies (all-SBUF operands, off the loaded DVE) -> ctxT f32r.
  6. out = ctx_norm @ Wo^T + bo_eff (bo_eff = bo + bv @ Wo^T).  The t<512
     half runs interleaved into the last pair's window-1 attention (its
     context is complete after window 0); bias add and output DMA are split
     512/256 across the sync/scalar queues to shorten the tail.
"""
import sys
import numpy as np

sys.path.insert(0, "/opt/trn_rl_repo")

import concourse.bass as bass
import concourse.mybir as mybir
import concourse.tile as tile

F32 = mybir.dt.float32
F32R = mybir.dt.float32r
BF16 = mybir.dt.bfloat16

B, T, E, H, D = 8, 1024, 768, 12, 64
NCH = E // 128          # 6 e-chunks
NTC = T // 128          # 8 t-chunks
NW = T // 512           # 2 q-windows
NEG = -1.0e9


def _split_excess_waits(nc, max_waits: int = 1):
    """walrus on this stack accepts at most one embedded sync-wait per
    instruction; peel extras onto wait-only NoOps on the same engine."""
    for func in nc.m.functions:
        for bb in func.blocks:
            insts = bb.instructions
            i = 0
            while i < len(insts):
                inst = insts[i]
                si = getattr(inst, "sync_info", None)
                if si is None or len(si.on_wait) <= max_waits:
                    i += 1
                    continue
                waits = list(si.on_wait)
                keep, extra = waits[:max_waits], waits[max_waits:]
                nops = []
                while extra:
                    chunk, extra = extra[:max_waits], extra[max_waits:]
                    nop = mybir.InstNoOp(
                        name=f"{inst.name}_ws{len(nops)}", ins=[], outs=[])
                    nop.engine = inst.engine
                    nop.sync_info = mybir.SyncInfo(on_wait=chunk, on_update=[])
                    nc.register_instruction(nop, overwrite=True)
                    nops.append(nop)
                si.on_wait = keep
                for j, nop in enumerate(nops):
                    insts.insert(i + j, nop)
                i += len(nops) + 1


def build_nc(repeat: int = 1):
    nc = bass.Bass()
    xT = nc.dram_tensor("xT", [E, T], F32, kind="ExternalInput")
    wq_r = nc.dram_tensor("wq_r", [NCH, 128, NCH, 128], F32, kind="ExternalInput")
    wk_r = nc.dram_tensor("wk_r", [NCH, 128, NCH, 128], F32, kind="ExternalInput")
    wvT = nc.dram_tensor("wvT", [E, E], F32, kind="ExternalInput")
    woT = nc.dram_tensor("woT", [E, E], F32, kind="ExternalInput")
    bq_pm = nc.dram_tensor("bq_pm", [128, NCH], F32, kind="ExternalInput")
    bk_pm = nc.dram_tensor("bk_pm", [128, NCH], F32, kind="ExternalInput")
    bo_bc = nc.dram_tensor("bo_bc", [128, E], F32, kind="ExternalInput")
    out = nc.dram_tensor("out", [T, E], F32, kind="ExternalOutput")

    tril = np.where(np.arange(128)[None, :] >= np.arange(128)[:, None],
                    0.0, NEG).astype(np.float32)
    maskc = nc.inline_tensor(tril, name="maskc")
    warmc = nc.inline_tensor(np.zeros((128, 128), np.float32), name="warmc")

    with tile.TileContext(nc) as tc:
        from contextlib import ExitStack
        with ExitStack() as ctx:
            consts = ctx.enter_context(tc.tile_pool(name="consts", bufs=1))
            persist = ctx.enter_context(tc.tile_pool(name="persist", bufs=1))
            wqk_p = ctx.enter_context(tc.tile_pool(name="wqk", bufs=4))
            wrow_p = ctx.enter_context(tc.tile_pool(name="wrow", bufs=6))
            pt_p = ctx.enter_context(tc.tile_pool(name="pt", bufs=3))
            rb_p = ctx.enter_context(tc.tile_pool(name="rb", bufs=6))
            cu_p = ctx.enter_context(tc.tile_pool(name="cu", bufs=6))
            out_p = ctx.enter_context(tc.tile_pool(name="outp", bufs=2))
            drp = ctx.enter_context(tc.tile_pool(name="drp", bufs=2, space="DRAM"))
            stp = ctx.enter_context(tc.tile_pool(name="stp", bufs=2, space="PSUM"))
            ctxp = ctx.enter_context(tc.tile_pool(name="ctxp", bufs=2, space="PSUM"))
            pp = ctx.enter_context(tc.tile_pool(name="pp", bufs=2, space="PSUM"))

            def body():
                mask_sb = consts.tile([128, 128], F32)
                warm_sb = consts.tile([128, 128], F32R)
                bqs = consts.tile([128, NCH], F32)
                bks = consts.tile([128, NCH], F32)
                bos = consts.tile([128, E], F32)

                xt_sb = persist.tile([128, NCH, T], F32R)
                qt_sb = persist.tile([128, NCH, T], F32R)
                kt_sb = persist.tile([128, NCH, T], F32R)
                v65_sb = persist.tile([128, NTC, H * 65], F32R)
                ctxT_sb = persist.tile([128, NCH, T], F32R)
                # reciprocal staging: dens at partitions 0 / 32 (quadrant-
                # aligned single-partition DVE writes), double-buffered
                rt_all = persist.tile([33, 2, 512], F32)

                # --- input / const DMAs ---
                # first-needed first: x chunk 0 + pair-0 weights + biases, so
                # the first projection matmul can start ~1.5us in.
                nc.scalar.dma_start(out=warm_sb, in_=warmc[:, :].bitcast(F32R))
                nc.sync.dma_start(out=xt_sb[:, 0, :],
                                  in_=xT[0:128, :].bitcast(F32R))
                wq_t0 = wqk_p.tile([128, NCH, 128], F32R, tag="wqk")
                nc.sync.dma_start(out=wq_t0, in_=wq_r[0, :, :, :].bitcast(F32R))
                nc.scalar.dma_start(out=bqs, in_=bq_pm[:, :])
                nc.scalar.dma_start(out=bks, in_=bk_pm[:, :])
                wk_t0 = wqk_p.tile([128, NCH, 128], F32R, tag="wqk")
                nc.scalar.dma_start(out=wk_t0, in_=wk_r[0, :, :, :].bitcast(F32R))
                for ch in range(1, NCH):
                    nc.sync.dma_start(out=xt_sb[:, ch, :],
                                      in_=xT[ch * 128:(ch + 1) * 128, :].bitcast(F32R))
                nc.scalar.dma_start(out=mask_sb, in_=maskc[:, :])
                nc.scalar.dma_start(out=bos, in_=bo_bc[:, :])
                wv_t = {}
                for ch in range(NCH):
                    w = wrow_p.tile([128, E], F32R, tag="wrow")
                    nc.sync.dma_start(
                        out=w, in_=wvT[ch * 128:(ch + 1) * 128, :].bitcast(F32R))
                    wv_t[ch] = w

                # PE warmup while input DMAs stream: ride out the cold-clock
                # window on throwaway matmuls so real work starts warm
                wps = pp.tile([128, 128], F32, tag="pp", name="wps")
                for _ in range(20):
                    nc.tensor.matmul(wps, warm_sb, warm_sb,
                                     start=True, stop=True)

                # ones columns of V65 (col 64 of every head), one strided memset
                v65_r = v65_sb.rearrange("p k (h e) -> p k h e", e=65)
                nc.vector.memset(v65_r[:, :, :, 64:65].bitcast(F32), 1.0)

                # --- PE work-item generators (projections as filler) ---
                def proj_qk_win(m, w_t, bias_sb, dst_sb, copy_eng, win):
                    """Yield thunks: 6 accumulating matmuls + 1 copy."""
                    ps = pp.tile([128, 512], F32, tag="pp", name=f"qk{m}w{win}")
                    for ch in range(NCH):
                        def mm(ps=ps, ch=ch, win=win):
                            nc.tensor.matmul(
                                ps, w_t[:, ch, :],
                                xt_sb[:, ch, win * 512:(win + 1) * 512],
                                start=(ch == 0), stop=(ch == NCH - 1))
                        yield mm
                    def cp(ps=ps, win=win):
                        dst = dst_sb[:, m, win * 512:(win + 1) * 512]
                        if copy_eng == "act":
                            nc.scalar.activation(
                                dst, ps,
                                mybir.ActivationFunctionType.Identity,
                                bias=bias_sb[:, m:m + 1])
                        else:
                            nc.vector.tensor_scalar_add(
                                dst, ps, bias_sb[:, m:m + 1])
                    yield cp

                def proj_qk_items(m, wq, wk):
                    """Q/K projections for pair m, ordered so window-0 Q and K
                    are both ready as early as possible."""
                    yield from proj_qk_win(m, wq, bqs, qt_sb, "dve", 0)
                    yield from proj_qk_win(m, wk, bks, kt_sb, "dve", 0)
                    yield from proj_qk_win(m, wq, bqs, qt_sb, "dve", 1)
                    yield from proj_qk_win(m, wk, bks, kt_sb, "dve", 1)

                def proj_v_items(kc):
                    ps0 = pp.tile([128, 512], F32, tag="pp")
                    ps1 = pp.tile([128, 256], F32, tag="pp")
                    for ch in range(NCH):
                        def mm(ps0=ps0, ps1=ps1, ch=ch, kc=kc):
                            lhsT = xt_sb[:, ch, kc * 128:(kc + 1) * 128]
                            nc.tensor.matmul(ps0, lhsT, wv_t[ch][:, 0:512],
                                             start=(ch == 0), stop=(ch == NCH - 1))
                            nc.tensor.matmul(ps1, lhsT, wv_t[ch][:, 512:768],
                                             start=(ch == 0), stop=(ch == NCH - 1))
                        yield mm
                    def cp(ps0=ps0, ps1=ps1, kc=kc):
                        v65r = v65_sb[:, kc, :].rearrange("p (h e) -> p h e", e=65)
                        nc.scalar.copy(v65r[:, 0:8, 0:64], ps0)
                        nc.scalar.copy(v65r[:, 8:12, 0:64], ps1)
                    yield cp

                def emit(it, n):
                    """Run up to n filler items; return #emitted."""
                    k = 0
                    for f in it:
                        f()
                        k += 1
                        if k >= n:
                            break
                    return k

                def attn_win(p, win, filler=iter(()), fpk=1):
                    """Attention for head pair p (heads 2p, 2p+1), q window win.
                    filler: iterator of PE work thunks interleaved per k-chunk."""
                    nk = 4 * (win + 1)
                    w0 = win * 512
                    hA, hB = 2 * p, 2 * p + 1
                    ctxA = ctxp.tile([65, 512], F32, tag="ctx")
                    ctxB = ctxp.tile([65, 512], F32, tag="ctx")
                    pend = None  # (kc, pt, off) awaiting PV
                    for kc in range(nk):
                        off = max(kc * 128 - w0, 0)
                        st = stp.tile([128, 1024], F32, tag="st")
                        diag = kc * 128 - w0 >= 0
                        soff = 256 if off == 384 else off
                        nc.tensor.matmul(
                            st[:, soff:512],
                            kt_sb[0:64, p, kc * 128:(kc + 1) * 128],
                            qt_sb[0:64, p, w0 + soff:w0 + 512],
                            start=True, stop=True, tile_position=(0, 0))
                        nc.tensor.matmul(
                            st[:, 512 + soff:1024],
                            kt_sb[64:128, p, kc * 128:(kc + 1) * 128],
                            qt_sb[64:128, p, w0 + soff:w0 + 512],
                            start=True, stop=True, tile_position=(64, 0))
                        if diag:
                            # one strided op masks both heads' diagonal
                            # blocks; the mask operand broadcasts via a
                            # free-dim stride-0 AP
                            st4 = st.rearrange("p (s q) -> p s q", s=2)
                            view = st4[:, :, off:off + 128]
                            m2 = mask_sb[:, :]
                            mask_b = bass.AP(
                                tensor=m2.tensor, offset=m2.offset,
                                ap=[list(m2.ap)[0], [0, 2], list(m2.ap)[1]])
                            nc.vector.tensor_tensor(
                                out=view, in0=view, in1=mask_b,
                                op=mybir.AluOpType.add)
                        # exp -> bf16 P^T (ACT)
                        pt = pt_p.tile([128, 1024], F32R, tag="pt")
                        if off > 0:
                            st3 = st.rearrange("p (s q) -> p s q", s=2)
                            pt3 = pt.rearrange("p (s q) -> p s q", s=2)
                            nc.scalar.activation(
                                pt3[:, :, off:512], st3[:, :, off:512],
                                mybir.ActivationFunctionType.Exp)
                        else:
                            nc.scalar.activation(
                                pt, st, mybir.ActivationFunctionType.Exp)
                        emit(filler, fpk)
                        if pend is not None:
                            pkc, ppt, poff = pend
                            nc.tensor.matmul(
                                ctxA[:, poff:512],
                                v65_sb[:, pkc, hA * 65:hA * 65 + 65],
                                ppt[:, poff:512],
                                start=(pkc == 0), stop=False)
                            nc.tensor.matmul(
                                ctxB[:, poff:512],
                                v65_sb[:, pkc, hB * 65:hB * 65 + 65],
                                ppt[:, 512 + poff:1024],
                                start=(pkc == 0), stop=False)
                        pend = (kc, pt, off)
                    # drain last PV
                    pkc, ppt, poff = pend
                    nc.tensor.matmul(
                        ctxA[:, poff:512],
                        v65_sb[:, pkc, hA * 65:hA * 65 + 65],
                        ppt[:, poff:512],
                        start=(pkc == 0), stop=True)
                    nc.tensor.matmul(
                        ctxB[:, poff:512],
                        v65_sb[:, pkc, hB * 65:hB * 65 + 65],
                        ppt[:, 512 + poff:1024],
                        start=(pkc == 0), stop=True)
                    emit(filler, 2)
                    # copy ctx PSUM -> SBUF immediately so the PSUM tiles
                    # free for the next window; normalize off the critical
                    # path: r = 1/den (DVE), DRAM-bounce broadcast (stride-0
                    # partition source), DVE mults (SBUF x SBUF, base 0)
                    cuA = cu_p.tile([65, 512], F32, tag="cu", name="cuA")
                    nc.scalar.copy(cuA, ctxA)
                    cuB = cu_p.tile([65, 512], F32, tag="cu", name="cuB")
                    nc.scalar.copy(cuB, ctxB)
                    rt = rt_all[:, (2 * p + win) % 2, :]
                    nc.vector.reciprocal(rt[0:1, :], cuA[64:65, :])
                    nc.vector.reciprocal(rt[32:33, :], cuB[64:65, :])
                    sc = drp.tile([2, 512], F32, tag="dr", name="sc")
                    nc.sync.dma_start(out=sc[0:1, :], in_=rt[0:1, :])
                    nc.sync.dma_start(out=sc[1:2, :], in_=rt[32:33, :])
                    rbA = rb_p.tile([64, 512], F32, tag="rb", name="rbA")
                    nc.scalar.dma_start(
                        out=rbA,
                        in_=bass.AP(tensor=sc.tensor, offset=sc[0:1, :].offset,
                                    ap=[[0, 64], [1, 512]]))
                    rbB = rb_p.tile([64, 512], F32, tag="rb", name="rbB")
                    nc.scalar.dma_start(
                        out=rbB,
                        in_=bass.AP(tensor=sc.tensor, offset=sc[1:2, :].offset,
                                    ap=[[0, 64], [1, 512]]))
                    # all-SBUF multiply runs on the otherwise-idle GPSIMD
                    nc.gpsimd.tensor_tensor(
                        out=ctxT_sb[0:64, p, w0:w0 + 512],
                        in0=cuA[0:64, :], in1=rbA[:, :],
                        op=mybir.AluOpType.mult)
                    nc.gpsimd.tensor_tensor(
                        out=ctxT_sb[64:128, p, w0:w0 + 512],
                        in0=cuB[0:64, :], in1=rbB[:, :],
                        op=mybir.AluOpType.mult)


                def outproj_items(tcns):
                    for tcn in tcns:
                        ps0 = pp.tile([128, 512], F32, tag="pp",
                                      name=f"ops0_{tcn}")
                        ps1 = pp.tile([128, 256], F32, tag="pp",
                                      name=f"ops1_{tcn}")
                        for ch in range(NCH):
                            def mm(ps0=ps0, ps1=ps1, ch=ch, tcn=tcn):
                                lhsT = ctxT_sb[:, ch, tcn * 128:(tcn + 1) * 128]
                                nc.tensor.matmul(
                                    ps0, lhsT, wo_t[ch][:, 0:512],
                                    start=(ch == 0), stop=(ch == NCH - 1))
                                nc.tensor.matmul(
                                    ps1, lhsT, wo_t[ch][:, 512:768],
                                    start=(ch == 0), stop=(ch == NCH - 1))
                            yield mm
                        def fin(ps0=ps0, ps1=ps1, tcn=tcn):
                            ot = out_p.tile([128, E], F32, tag="outp")
                            nc.vector.tensor_tensor(
                                out=ot[:, 0:512], in0=ps0, in1=bos[:, 0:512],
                                op=mybir.AluOpType.add)
                            nc.sync.dma_start(
                                out=out[tcn * 128:(tcn + 1) * 128, 0:512],
                                in_=ot[:, 0:512])
                            nc.vector.tensor_tensor(
                                out=ot[:, 512:768], in0=ps1,
                                in1=bos[:, 512:768], op=mybir.AluOpType.add)
                            nc.scalar.dma_start(
                                out=out[tcn * 128:(tcn + 1) * 128, 512:768],
                                in_=ot[:, 512:768])
                        yield fin

                # --- emission schedule ---
                # Pair 0 Q/K projections up front (no filler context yet).
                for f in proj_qk_items(0, wq_t0, wk_t0):
                    f()
                # prefetch pair-1 weights
                wq_t1 = wqk_p.tile([128, NCH, 128], F32R, tag="wqk")
                nc.sync.dma_start(out=wq_t1, in_=wq_r[1, :, :, :].bitcast(F32R))
                wk_t1 = wqk_p.tile([128, NCH, 128], F32R, tag="wqk")
                nc.sync.dma_start(out=wk_t1, in_=wk_r[1, :, :, :].bitcast(F32R))
                # V projection for k-chunks 0..3 (needed by win 0)
                for kc in range(4):
                    for f in proj_v_items(kc):
                        f()

                def chain(*gens):
                    for g in gens:
                        yield from g

                # attn(0, win0) with V k-chunks 4..7 as filler (2 items/kc)
                fill = chain(*[proj_v_items(kc) for kc in range(4, NTC)])
                attn_win(0, 0, fill, fpk=2)
                emit(fill, 100)  # drain any leftover V items
                # attn(0, win1) with pair-1 Q/K proj as filler
                fill = proj_qk_items(1, wq_t1, wk_t1)
                attn_win(0, 1, fill, fpk=2)
                emit(fill, 100)

                wq_t = {0: wq_t0, 1: wq_t1}
                wk_t = {0: wk_t0, 1: wk_t1}
                wo_t = {}
                for p in range(1, NCH):
                    # prefetch next pair's weights / wo rows
                    if p + 1 < NCH:
                        wq_t[p + 1] = wqk_p.tile([128, NCH, 128], F32R, tag="wqk", name=f"wq_t{p+1}")
                        nc.sync.dma_start(out=wq_t[p + 1],
                                          in_=wq_r[p + 1, :, :, :].bitcast(F32R))
                        wk_t[p + 1] = wqk_p.tile([128, NCH, 128], F32R, tag="wqk", name=f"wk_t{p+1}")
                        nc.sync.dma_start(out=wk_t[p + 1],
                                          in_=wk_r[p + 1, :, :, :].bitcast(F32R))
                    else:
                        for ch in range(NCH):
                            w = wrow_p.tile([128, E], F32R, tag="wrow")
                            nc.sync.dma_start(
                                out=w,
                                in_=woT[ch * 128:(ch + 1) * 128, :].bitcast(F32R))
                            wo_t[ch] = w
                    if p + 1 < NCH:
                        fill = proj_qk_items(p + 1, wq_t[p + 1], wk_t[p + 1])
                        attn_win(p, 0, fill, fpk=2)
                        attn_win(p, 1, fill, fpk=2)
                        emit(fill, 100)
                    else:
                        # last pair: window-1 PE gaps are filled with the
                        # t-chunks of the output projection that only need
                        # this pair's window-0 context (t < 512)
                        attn_win(p, 0, iter(()))
                        fill = outproj_items(range(4))
                        attn_win(p, 1, fill, fpk=2)
                        emit(fill, 100)

                # --- output projection, second half ---
                for f in outproj_items(range(4, NTC)):
                    f()

            for _rep in range(repeat):
                body()

    _split_excess_waits(nc)
    return nc


_NC_CACHE = None


def _make_in_maps(x, Wq, bq, Wk, bk, Wv, bv, Wo, bo):
    scale = 1.0 / np.sqrt(D)

    def reorder(w):
        return np.ascontiguousarray(
            w.T.reshape(NCH, 128, NCH, 128).transpose(2, 1, 0, 3))

    wq_r = reorder((Wq.astype(np.float64) * scale).astype(np.float32))
    wk_r = reorder(Wk.astype(np.float32))
    wvT = np.ascontiguousarray(Wv.T.astype(np.float32))
    woT = np.ascontiguousarray(Wo.T.astype(np.float32))
    bq_pm = np.ascontiguousarray(
        (bq.astype(np.float64) * scale).astype(np.float32).reshape(NCH, 128).T)
    bk_pm = np.ascontiguousarray(bk.reshape(NCH, 128).T.astype(np.float32))
    bo_eff = (bo.astype(np.float64)
              + bv.astype(np.float64) @ Wo.T.astype(np.float64)).astype(np.float32)
    bo_bc = np.ascontiguousarray(np.tile(bo_eff[None, :], (128, 1)))
    maps = []
    for b in range(B):
        xTb = np.ascontiguousarray(x[b].T.astype(np.float32))
        maps.append({"xT": xTb, "wq_r": wq_r, "wk_r": wk_r, "wvT": wvT,
                     "woT": woT, "bq_pm": bq_pm, "bk_pm": bk_pm, "bo_bc": bo_bc})
    return maps


def kernel(x, Wq, bq, Wk, bk, Wv, bv, Wo, bo):
    global _NC_CACHE
    from concourse.bass_utils import run_bass_kernel_spmd
    if _NC_CACHE is None:
        _NC_CACHE = build_nc()
    in_maps = _make_in_maps(x, Wq, bq, Wk, bk, Wv, bv, Wo, bo)
    res = run_bass_kernel_spmd(_NC_CACHE, in_maps, core_ids=list(range(B)))
    return np.stack([res.results[i]["out"] for i in range(B)], axis=0)



# revision 19
# speedup vs baseline: 1.1588x; 1.1588x over previous
"""Causal multi-head attention (B=8, T=1024, E=768, H=12, D=64) on 8 trn2
NeuronCores, data-parallel over the batch (one batch element per core).

v3 — f32r matmul datapath (bf16 emits LDWEIGHTS-per-matmul which serializes
on this walrus build) with fine-grained projection/attention interleaving.

Per-core pipeline (all matmuls float32r — full PE rate, ~1e-4 rel err):
  1. PE warmup matmuls ride out the cold-clock window while x/W DMAs stream
     (first-needed-first DMA order across the sync + scalar DGE queues).
  2. Q^T = (Wq/8) @ x^T + bq/8, K^T = Wk @ x^T + bk -> SBUF f32r [768, 1024]
     (softmax 1/sqrt(D) scale folded into Wq/bq on the host).
  3. V = x @ Wv^T (bias folded into the output projection bias) -> SBUF
     [1024, 12*65] with a ones column per head ("V65") so the attention
     context matmul also produces the softmax denominator.
  4. Per head pair, per 512-wide q window: S^T[k,q] via row-packed matmuls
     (K=64, tile_position (0,0)/(64,0)); the 128-wide triangle-edge tiles
     are widened to 256 into the never-read masked zone to dodge the f32r
     sub-256 4-cycle/row penalty; causal mask added on both heads'
     diagonal 128-blocks by one strided DVE op (free-dim stride-0 mask
     broadcast), exp on ACT -> f32r P^T, ctx^T[65,q] accumulated over
     k-chunks with V65 stationary.  Projection matmuls for the NEXT head
     pair are emitted between the k-chunk steps as PE filler so exp latency
     never stalls the in-order PE queue.
  5. Normalization: ctx is copied PSUM->SBUF on ACT (its window-boundary
     lull) right after the last PV so the PSUM accumulators free for the
     next window without queuing behind DVE; then (off the critical path) reciprocal of row 64 on DVE (partitions 0/32), broadcast across
     partitions via a DRAM-bounce DMA (stride-0 partition source; a PE
     rank-2 broadcast matmul measures ~32us on this HW), and GPSIMD
     multiplies (all-SBUF operands, off the loaded DVE) -> ctxT f32r.
  6. out = ctx_norm @ Wo^T + bo_eff (bo_eff = bo + bv @ Wo^T).  The t<512
     half runs interleaved into the last pair's window-1 attention (its
     context is complete after window 0); bias add and output DMA are split
     512/256 across the sync/scalar queues to shorten the tail.
"""
import sys
import numpy as np

sys.path.insert(0, "/opt/trn_rl_repo")

import concourse.bass as bass
import concourse.mybir as mybir
import concourse.tile as tile

F32 = mybir.dt.float32
F32R = mybir.dt.float32r
BF16 = mybir.dt.bfloat16

B, T, E, H, D = 8, 1024, 768, 12, 64
NCH = E // 128          # 6 e-chunks
NTC = T // 128          # 8 t-chunks
NW = T // 512           # 2 q-windows
NEG = -1.0e9


def _split_excess_waits(nc, max_waits: int = 1):
    """walrus on this stack accepts at most one embedded sync-wait per
    instruction; peel extras onto wait-only NoOps on the same engine."""
    for func in nc.m.functions:
        for bb in func.blocks:
            insts = bb.instructions
            i = 0
            while i < len(insts):
                inst = insts[i]
                si = getattr(inst, "sync_info", None)
                if si is None or len(si.on_wait) <= max_waits:
                    i += 1
                    continue
                waits = list(si.on_wait)
                keep, extra = waits[:max_waits], waits[max_waits:]
                nops = []
                while extra:
                    chunk, extra = extra[:max_waits], extra[max_waits:]
                    nop = mybir.InstNoOp(
                        name=f"{inst.name}_ws{len(nops)}", ins=[], outs=[])
                    nop.engine = inst.engine
                    nop.sync_info = mybir.SyncInfo(on_wait=chunk, on_update=[])
                    nc.register_instruction(nop, overwrite=True)
                    nops.append(nop)
                si.on_wait = keep
                for j, nop in enumerate(nops):
                    insts.insert(i + j, nop)
                i += len(nops) + 1


def build_nc(repeat: int = 1):
    nc = bass.Bass()
    xT = nc.dram_tensor("xT", [E, T], F32, kind="ExternalInput")
    wq_r = nc.dram_tensor("wq_r", [NCH, 128, NCH, 128], F32, kind="ExternalInput")
    wk_r = nc.dram_tensor("wk_r", [NCH, 128, NCH, 128], F32, kind="ExternalInput")
    wvT = nc.dram_tensor("wvT", [E, E], F32, kind="ExternalInput")
    woT = nc.dram_tensor("woT", [E, E], F32, kind="ExternalInput")
    bq_pm = nc.dram_tensor("bq_pm", [128, NCH], F32, kind="ExternalInput")
    bk_pm = nc.dram_tensor("bk_pm", [128, NCH], F32, kind="ExternalInput")
    bo_bc = nc.dram_tensor("bo_bc", [128, E], F32, kind="ExternalInput")
    out = nc.dram_tensor("out", [T, E], F32, kind="ExternalOutput")

    tril = np.where(np.arange(128)[None, :] >= np.arange(128)[:, None],
                    0.0, NEG).astype(np.float32)
    maskc = nc.inline_tensor(tril, name="maskc")
    warmc = nc.inline_tensor(np.zeros((128, 128), np.float32), name="warmc")

    with tile.TileContext(nc) as tc:
        from contextlib import ExitStack
        with ExitStack() as ctx:
            consts = ctx.enter_context(tc.tile_pool(name="consts", bufs=1))
            persist = ctx.enter_context(tc.tile_pool(name="persist", bufs=1))
            wqk_p = ctx.enter_context(tc.tile_pool(name="wqk", bufs=4))
            wrow_p = ctx.enter_context(tc.tile_pool(name="wrow", bufs=6))
            pt_p = ctx.enter_context(tc.tile_pool(name="pt", bufs=3))
            rb_p = ctx.enter_context(tc.tile_pool(name="rb", bufs=6))
            cu_p = ctx.enter_context(tc.tile_pool(name="cu", bufs=6))
            out_p = ctx.enter_context(tc.tile_pool(name="outp", bufs=4))
            drp = ctx.enter_context(tc.tile_pool(name="drp", bufs=2, space="DRAM"))

            def body():
                # PSUM pools are body-local: attention's pools (stp/ctxp/pp)
                # release before the tail so the final out-projection gets a
                # deep dedicated ring in the freed banks.
                psctx = ExitStack()
                pp = psctx.enter_context(
                    tc.tile_pool(name="pp", bufs=2, space="PSUM"))
                stp = psctx.enter_context(
                    tc.tile_pool(name="stp", bufs=2, space="PSUM"))
                ctxp = psctx.enter_context(
                    tc.tile_pool(name="ctxp", bufs=2, space="PSUM"))
                mask_sb = consts.tile([128, 128], F32)
                warm_sb = consts.tile([128, 128], F32R)
                bqs = consts.tile([128, NCH], F32)
                bks = consts.tile([128, NCH], F32)
                bos = consts.tile([128, E], F32)

                xt_sb = persist.tile([128, NCH, T], F32R)
                qt_sb = persist.tile([128, NCH, T], F32R)
                kt_sb = persist.tile([128, NCH, T], F32R)
                v65_sb = persist.tile([128, NTC, H * 65], F32R)
                ctxT_sb = persist.tile([128, NCH, T], F32R)
                # reciprocal staging: dens at partitions 0 / 32 (quadrant-
                # aligned single-partition DVE writes), double-buffered
                rt_all = persist.tile([33, 2, 512], F32)

                # --- input / const DMAs ---
                # first-needed first, alternating the sync/scalar HWDGE rings
                # so x chunks land every ~0.8us instead of serializing on one
                # FIFO ring.
                nc.vector.memset(warm_sb.bitcast(F32), 0.0)
                nc.sync.dma_start(out=xt_sb[:, 0, :],
                                  in_=xT[0:128, :].bitcast(F32R))
                nc.scalar.dma_start(out=xt_sb[:, 1, :],
                                    in_=xT[128:256, :].bitcast(F32R))
                wq_t0 = wqk_p.tile([128, NCH, 128], F32R, tag="wqk")
                nc.sync.dma_start(out=wq_t0, in_=wq_r[0, :, :, :].bitcast(F32R))
                wk_t0 = wqk_p.tile([128, NCH, 128], F32R, tag="wqk")
                nc.scalar.dma_start(out=wk_t0, in_=wk_r[0, :, :, :].bitcast(F32R))
                for ch in range(2, NCH):
                    eng = nc.sync if ch % 2 == 0 else nc.scalar
                    eng.dma_start(out=xt_sb[:, ch, :],
                                  in_=xT[ch * 128:(ch + 1) * 128, :].bitcast(F32R))
                nc.scalar.dma_start(out=bqs, in_=bq_pm[:, :])
                nc.scalar.dma_start(out=bks, in_=bk_pm[:, :])
                nc.sync.dma_start(out=mask_sb, in_=maskc[:, :])
                nc.scalar.dma_start(out=bos, in_=bo_bc[:, :])
                wv_t = {}
                for ch in range(NCH):
                    w = wrow_p.tile([128, E], F32R, tag="wrow")
                    eng = nc.sync if ch % 2 == 0 else nc.scalar
                    eng.dma_start(
                        out=w, in_=wvT[ch * 128:(ch + 1) * 128, :].bitcast(F32R))
                    wv_t[ch] = w

                # PE warmup while input DMAs stream: ride out the cold-clock
                # window on throwaway matmuls so real work starts warm
                wps = pp.tile([128, 128], F32, tag="pp", name="wps")
                for _ in range(20):
                    nc.tensor.matmul(wps, warm_sb, warm_sb,
                                     start=True, stop=True)
                # prime the ACT exp table during the DMA phase so the first
                # real exp doesn't pay the table-load latency
                expw = consts.tile([1, 8], F32)
                nc.scalar.activation(expw, warm_sb.bitcast(F32)[0:1, 0:8],
                                     mybir.ActivationFunctionType.Exp)

                # ones columns of V65 (col 64 of every head), one strided memset
                v65_r = v65_sb.rearrange("p k (h e) -> p k h e", e=65)
                nc.vector.memset(v65_r[:, :, :, 64:65].bitcast(F32), 1.0)

                # --- PE work-item generators (projections as filler) ---
                def proj_qk_win(m, w_t, bias_sb, dst_sb, copy_eng, win):
                    """Yield thunks: 6 accumulating matmuls + 1 copy."""
                    ps = pp.tile([128, 512], F32, tag="pp", name=f"qk{m}w{win}")
                    for ch in range(NCH):
                        def mm(ps=ps, ch=ch, win=win):
                            nc.tensor.matmul(
                                ps, w_t[:, ch, :],
                                xt_sb[:, ch, win * 512:(win + 1) * 512],
                                start=(ch == 0), stop=(ch == NCH - 1))
                        yield mm
                    def cp(ps=ps, win=win):
                        dst = dst_sb[:, m, win * 512:(win + 1) * 512]
                        if copy_eng == "act":
                            nc.scalar.activation(
                                dst, ps,
                                mybir.ActivationFunctionType.Identity,
                                bias=bias_sb[:, m:m + 1])
                        else:
                            nc.vector.tensor_scalar_add(
                                dst, ps, bias_sb[:, m:m + 1])
                    yield cp

                def proj_qk_items(m, wq, wk):
                    """Q/K projections for pair m, ordered so window-0 Q and K
                    are both ready as early as possible."""
                    yield from proj_qk_win(m, wq, bqs, qt_sb, "dve", 0)
                    yield from proj_qk_win(m, wk, bks, kt_sb, "dve", 0)
                    yield from proj_qk_win(m, wq, bqs, qt_sb, "dve", 1)
                    yield from proj_qk_win(m, wk, bks, kt_sb, "dve", 1)

                def proj_v_items(kc):
                    ps0 = pp.tile([128, 512], F32, tag="pp")
                    ps1 = pp.tile([128, 256], F32, tag="pp")
                    for ch in range(NCH):
                        def mm(ps0=ps0, ps1=ps1, ch=ch, kc=kc):
                            lhsT = xt_sb[:, ch, kc * 128:(kc + 1) * 128]
                            nc.tensor.matmul(ps0, lhsT, wv_t[ch][:, 0:512],
                                             start=(ch == 0), stop=(ch == NCH - 1))
                            nc.tensor.matmul(ps1, lhsT, wv_t[ch][:, 512:768],
                                             start=(ch == 0), stop=(ch == NCH - 1))
                        yield mm
                    def cp(ps0=ps0, ps1=ps1, kc=kc):
                        v65r = v65_sb[:, kc, :].rearrange("p (h e) -> p h e", e=65)
                        nc.scalar.copy(v65r[:, 0:8, 0:64], ps0)
                        nc.scalar.copy(v65r[:, 8:12, 0:64], ps1)
                    yield cp

                def emit(it, n):
                    """Run up to n filler items; return #emitted."""
                    k = 0
                    for f in it:
                        f()
                        k += 1
                        if k >= n:
                            break
                    return k

                def attn_win(p, win, filler=iter(()), fpk=1):
                    """Attention for head pair p (heads 2p, 2p+1), q window win.
                    filler: iterator of PE work thunks interleaved per k-chunk."""
                    nk = 4 * (win + 1)
                    w0 = win * 512
                    hA, hB = 2 * p, 2 * p + 1
                    ctxA = ctxp.tile([65, 512], F32, tag="ctx")
                    ctxB = ctxp.tile([65, 512], F32, tag="ctx")
                    pend = None  # (kc, pt, off) awaiting PV
                    for kc in range(nk):
                        off = max(kc * 128 - w0, 0)
                        st = stp.tile([128, 1024], F32, tag="st")
                        diag = kc * 128 - w0 >= 0
                        soff = 256 if off == 384 else off
                        nc.tensor.matmul(
                            st[:, soff:512],
                            kt_sb[0:64, p, kc * 128:(kc + 1) * 128],
                            qt_sb[0:64, p, w0 + soff:w0 + 512],
                            start=True, stop=True, tile_position=(0, 0))
                        nc.tensor.matmul(
                            st[:, 512 + soff:1024],
                            kt_sb[64:128, p, kc * 128:(kc + 1) * 128],
                            qt_sb[64:128, p, w0 + soff:w0 + 512],
                            start=True, stop=True, tile_position=(64, 0))
                        if diag:
                            # one strided op masks both heads' diagonal
                            # blocks; the mask operand broadcasts via a
                            # free-dim stride-0 AP
                            st4 = st.rearrange("p (s q) -> p s q", s=2)
                            view = st4[:, :, off:off + 128]
                            m2 = mask_sb[:, :]
                            mask_b = bass.AP(
                                tensor=m2.tensor, offset=m2.offset,
                                ap=[list(m2.ap)[0], [0, 2], list(m2.ap)[1]])
                            nc.vector.tensor_tensor(
                                out=view, in0=view, in1=mask_b,
                                op=mybir.AluOpType.add)
                        # exp -> bf16 P^T (ACT)
                        pt = pt_p.tile([128, 1024], F32R, tag="pt")
                        if off > 0:
                            st3 = st.rearrange("p (s q) -> p s q", s=2)
                            pt3 = pt.rearrange("p (s q) -> p s q", s=2)
                            nc.scalar.activation(
                                pt3[:, :, off:512], st3[:, :, off:512],
                                mybir.ActivationFunctionType.Exp)
                        else:
                            nc.scalar.activation(
                                pt, st, mybir.ActivationFunctionType.Exp)
                        emit(filler, fpk)
                        if pend is not None:
                            pkc, ppt, poff = pend
                            nc.tensor.matmul(
                                ctxA[:, poff:512],
                                v65_sb[:, pkc, hA * 65:hA * 65 + 65],
                                ppt[:, poff:512],
                                start=(pkc == 0), stop=False)
                            nc.tensor.matmul(
                                ctxB[:, poff:512],
                                v65_sb[:, pkc, hB * 65:hB * 65 + 65],
                                ppt[:, 512 + poff:1024],
                                start=(pkc == 0), stop=False)
                        pend = (kc, pt, off)
                    # drain last PV
                    pkc, ppt, poff = pend
                    nc.tensor.matmul(
                        ctxA[:, poff:512],
                        v65_sb[:, pkc, hA * 65:hA * 65 + 65],
                        ppt[:, poff:512],
                        start=(pkc == 0), stop=True)
                    nc.tensor.matmul(
                        ctxB[:, poff:512],
                        v65_sb[:, pkc, hB * 65:hB * 65 + 65],
                        ppt[:, 512 + poff:1024],
                        start=(pkc == 0), stop=True)
                    emit(filler, 2)
                    # Normalize: r = 1/den (DVE, straight from the PSUM den
                    # row), broadcast r across 64 partitions with GPSIMD's
                    # native partition_broadcast (no DRAM bounce), then
                    # head A multiplies PSUM ctx directly on DVE while head B
                    # goes ACT-copy -> GPSIMD multiply so the two heads'
                    # normalize work lands on different engines.
                    # reciprocals straight from the PSUM den rows; one
                    # combined [2,512] DRAM stage (partitions {0,32}) and one
                    # combined broadcast read into a [128,512] tile (rows
                    # 0-63 <- head A recip, 64-127 <- head B recip).
                    rt = rt_all[:, (2 * p + win) % 2, :]
                    nc.vector.reciprocal(rt[0:1, :], ctxA[64:65, :])
                    nc.vector.reciprocal(rt[32:33, :], ctxB[64:65, :])
                    sc = drp.tile([2, 512], F32, tag="dr", name="sc")
                    nc.sync.dma_start(out=sc[0:1, :], in_=rt[0:1, :])
                    nc.sync.dma_start(out=sc[1:2, :], in_=rt[32:33, :])
                    cuA = cu_p.tile([64, 512], F32, tag="cu", name="cuA")
                    nc.vector.tensor_copy(cuA, ctxA[0:64, :])
                    cuB = cu_p.tile([64, 512], F32, tag="cu", name="cuB")
                    nc.scalar.copy(cuB, ctxB[0:64, :])
                    rbA = rb_p.tile([64, 512], F32, tag="rb", name="rbA")
                    nc.gpsimd.dma_start(
                        out=rbA,
                        in_=bass.AP(tensor=sc.tensor, offset=sc[0:1, :].offset,
                                    ap=[[0, 64], [1, 512]]))
                    rbB = rb_p.tile([64, 512], F32, tag="rb", name="rbB")
                    nc.gpsimd.dma_start(
                        out=rbB,
                        in_=bass.AP(tensor=sc.tensor, offset=sc[1:2, :].offset,
                                    ap=[[0, 64], [1, 512]]))
                    nc.gpsimd.tensor_tensor(
                        out=ctxT_sb[0:64, p, w0:w0 + 512],
                        in0=cuA[:, :], in1=rbA[:, :],
                        op=mybir.AluOpType.mult)
                    nc.gpsimd.tensor_tensor(
                        out=ctxT_sb[64:128, p, w0:w0 + 512],
                        in0=cuB[:, :], in1=rbB[:, :],
                        op=mybir.AluOpType.mult)


                def outproj_items(tcns, pool0=None, tag0="pp", tag1="pp"):
                    for tcn in tcns:
                        pl = pool0 if pool0 is not None else pp
                        ps0 = pl.tile([128, 512], F32, tag=tag0,
                                      name=f"ops0_{tcn}")
                        ps1 = pl.tile([128, 256], F32, tag=tag1,
                                      name=f"ops1_{tcn}")
                        for ch in range(NCH):
                            def mm(ps0=ps0, ps1=ps1, ch=ch, tcn=tcn):
                                lhsT = ctxT_sb[:, ch, tcn * 128:(tcn + 1) * 128]
                                nc.tensor.matmul(
                                    ps0, lhsT, wo_t[ch][:, 0:512],
                                    start=(ch == 0), stop=(ch == NCH - 1))
                                nc.tensor.matmul(
                                    ps1, lhsT, wo_t[ch][:, 512:768],
                                    start=(ch == 0), stop=(ch == NCH - 1))
                            yield mm
                        def fin(ps0=ps0, ps1=ps1, tcn=tcn):
                            # 3x256-wide bias+store pieces on three DMA
                            # queues so the final drain is one short piece
                            ot = out_p.tile([128, E], F32, tag="outp")
                            nc.vector.tensor_tensor(
                                out=ot[:, 0:256], in0=ps0[:, 0:256],
                                in1=bos[:, 0:256], op=mybir.AluOpType.add)
                            nc.sync.dma_start(
                                out=out[tcn * 128:(tcn + 1) * 128, 0:256],
                                in_=ot[:, 0:256])
                            nc.vector.tensor_tensor(
                                out=ot[:, 256:512], in0=ps0[:, 256:512],
                                in1=bos[:, 256:512], op=mybir.AluOpType.add)
                            nc.scalar.dma_start(
                                out=out[tcn * 128:(tcn + 1) * 128, 256:512],
                                in_=ot[:, 256:512])
                            nc.vector.tensor_tensor(
                                out=ot[:, 512:768], in0=ps1,
                                in1=bos[:, 512:768], op=mybir.AluOpType.add)
                            nc.gpsimd.dma_start(
                                out=out[tcn * 128:(tcn + 1) * 128, 512:768],
                                in_=ot[:, 512:768])
                        yield fin

                # --- emission schedule ---
                # Pair 0 Q/K projections up front (no filler context yet).
                for f in proj_qk_items(0, wq_t0, wk_t0):
                    f()
                # prefetch pair-1 weights
                wq_t1 = wqk_p.tile([128, NCH, 128], F32R, tag="wqk")
                nc.sync.dma_start(out=wq_t1, in_=wq_r[1, :, :, :].bitcast(F32R))
                wk_t1 = wqk_p.tile([128, NCH, 128], F32R, tag="wqk")
                nc.sync.dma_start(out=wk_t1, in_=wk_r[1, :, :, :].bitcast(F32R))
                # V projection for k-chunks 0..3 (needed by win 0)
                for kc in range(4):
                    for f in proj_v_items(kc):
                        f()

                def chain(*gens):
                    for g in gens:
                        yield from g

                # attn(0, win0) with V k-chunks 4..7 as filler (2 items/kc)
                fill = chain(*[proj_v_items(kc) for kc in range(4, NTC)])
                attn_win(0, 0, fill, fpk=2)
                emit(fill, 100)  # drain any leftover V items
                # attn(0, win1) with pair-1 Q/K proj as filler
                fill = proj_qk_items(1, wq_t1, wk_t1)
                attn_win(0, 1, fill, fpk=2)
                emit(fill, 100)

                wq_t = {0: wq_t0, 1: wq_t1}
                wk_t = {0: wk_t0, 1: wk_t1}
                wo_t = {}
                for p in range(1, NCH):
                    # prefetch next pair's weights / wo rows
                    if p + 1 < NCH:
                        wq_t[p + 1] = wqk_p.tile([128, NCH, 128], F32R, tag="wqk", name=f"wq_t{p+1}")
                        nc.sync.dma_start(out=wq_t[p + 1],
                                          in_=wq_r[p + 1, :, :, :].bitcast(F32R))
                        wk_t[p + 1] = wqk_p.tile([128, NCH, 128], F32R, tag="wqk", name=f"wk_t{p+1}")
                        nc.sync.dma_start(out=wk_t[p + 1],
                                          in_=wk_r[p + 1, :, :, :].bitcast(F32R))
                    else:
                        for ch in range(NCH):
                            w = wrow_p.tile([128, E], F32R, tag="wrow")
                            nc.sync.dma_start(
                                out=w,
                                in_=woT[ch * 128:(ch + 1) * 128, :].bitcast(F32R))
                            wo_t[ch] = w
                    if p + 1 < NCH:
                        fill = proj_qk_items(p + 1, wq_t[p + 1], wk_t[p + 1])
                        attn_win(p, 0, fill, fpk=2)
                        attn_win(p, 1, fill, fpk=2)
                        emit(fill, 100)
                    else:
                        # last pair: window-1 PE gaps are filled with the
                        # t-chunks of the output projection that only need
                        # this pair's window-0 context (t < 512)
                        attn_win(p, 0, iter(()))
                        fill = outproj_items(range(4))
                        attn_win(p, 1, fill, fpk=2)
                        emit(fill, 100)

                # --- output projection, second half ---
                # attention's PSUM pools release here; the tail runs out of a
                # dedicated 4-deep ring so all four t-chunks pipeline.
                psctx.close()
                with tc.tile_pool(name="po", bufs=4, space="PSUM") as po:
                    for f in outproj_items(range(4, NTC), po, "po0", "po1"):
                        f()

            for _rep in range(repeat):
                body()

    _split_excess_waits(nc)
    return nc


_NC_CACHE = None


def _make_in_maps(x, Wq, bq, Wk, bk, Wv, bv, Wo, bo):
    scale = 1.0 / np.sqrt(D)

    def reorder(w):
        return np.ascontiguousarray(
            w.T.reshape(NCH, 128, NCH, 128).transpose(2, 1, 0, 3))

    wq_r = reorder((Wq.astype(np.float64) * scale).astype(np.float32))
    wk_r = reorder(Wk.astype(np.float32))
    wvT = np.ascontiguousarray(Wv.T.astype(np.float32))
    woT = np.ascontiguousarray(Wo.T.astype(np.float32))
    bq_pm = np.ascontiguousarray(
        (bq.astype(np.float64) * scale).astype(np.float32).reshape(NCH, 128).T)
    bk_pm = np.ascontiguousarray(bk.reshape(NCH, 128).T.astype(np.float32))
    bo_eff = (bo.astype(np.float64)
              + bv.astype(np.float64) @ Wo.T.astype(np.float64)).astype(np.float32)
    bo_bc = np.ascontiguousarray(np.tile(bo_eff[None, :], (128, 1)))
    maps = []
    for b in range(B):
        xTb = np.ascontiguousarray(x[b].T.astype(np.float32))
        maps.append({"xT": xTb, "wq_r": wq_r, "wk_r": wk_r, "wvT": wvT,
                     "woT": woT, "bq_pm": bq_pm, "bk_pm": bk_pm, "bo_bc": bo_bc})
    return maps


def kernel(x, Wq, bq, Wk, bk, Wv, bv, Wo, bo):
    global _NC_CACHE
    from concourse.bass_utils import run_bass_kernel_spmd
    if _NC_CACHE is None:
        _NC_CACHE = build_nc()
    in_maps = _make_in_maps(x, Wq, bq, Wk, bk, Wv, bv, Wo, bo)
    res = run_bass_kernel_spmd(_NC_CACHE, in_maps, core_ids=list(range(B)))
    return np.stack([res.results[i]["out"] for i in range(B)], axis=0)



# revision 24
# speedup vs baseline: 1.1980x; 1.0338x over previous
"""Causal multi-head attention (B=8, T=1024, E=768, H=12, D=64) on 8 trn2
NeuronCores, data-parallel over the batch (one batch element per core).

v3 — f32r matmul datapath (bf16 emits LDWEIGHTS-per-matmul which serializes
on this walrus build) with fine-grained projection/attention interleaving.

Per-core pipeline (all matmuls float32r — full PE rate, ~1e-4 rel err):
  1. PE warmup matmuls ride out the cold-clock window while x/W DMAs stream
     (first-needed-first DMA order across the sync + scalar DGE queues).
  2. Q^T = (Wq/8) @ x^T + bq/8, K^T = Wk @ x^T + bk -> SBUF f32r [768, 1024]
     (softmax 1/sqrt(D) scale folded into Wq/bq on the host).
  3. V = x @ Wv^T (bias folded into the output projection bias) -> SBUF
     [1024, 12*65] with a ones column per head ("V65") so the attention
     context matmul also produces the softmax denominator.
  4. Per head pair, per 512-wide q window: S^T[k,q] via row-packed matmuls
     (K=64, tile_position (0,0)/(64,0)); the 128-wide triangle-edge tiles
     are widened to 256 into the never-read masked zone to dodge the f32r
     sub-256 4-cycle/row penalty; causal mask added on both heads'
     diagonal 128-blocks by one strided DVE op (free-dim stride-0 mask
     broadcast), exp on ACT -> f32r P^T, ctx^T[65,q] accumulated over
     k-chunks with V65 stationary.  Projection matmuls for the NEXT head
     pair are emitted between the k-chunk steps as PE filler so exp latency
     never stalls the in-order PE queue.
  5. Normalization: ctx is copied PSUM->SBUF on ACT (its window-boundary
     lull) right after the last PV so the PSUM accumulators free for the
     next window without queuing behind DVE; then (off the critical path) reciprocal of row 64 on DVE (partitions 0/32), broadcast across
     partitions via a DRAM-bounce DMA (stride-0 partition source; a PE
     rank-2 broadcast matmul measures ~32us on this HW), and GPSIMD
     multiplies (all-SBUF operands, off the loaded DVE) -> ctxT f32r.
  6. out = ctx_norm @ Wo^T + bo_eff (bo_eff = bo + bv @ Wo^T).  The t<512
     half runs interleaved into the last pair's window-1 attention (its
     context is complete after window 0); bias add and output DMA are split
     512/256 across the sync/scalar queues to shorten the tail.
"""
import sys
import numpy as np

sys.path.insert(0, "/opt/trn_rl_repo")

import concourse.bass as bass
import concourse.mybir as mybir
import concourse.tile as tile

F32 = mybir.dt.float32
F32R = mybir.dt.float32r
BF16 = mybir.dt.bfloat16

B, T, E, H, D = 8, 1024, 768, 12, 64
NCH = E // 128          # 6 e-chunks
NTC = T // 128          # 8 t-chunks
NW = T // 512           # 2 q-windows
NEG = -1.0e9


def _split_excess_waits(nc, max_waits: int = 1):
    """walrus on this stack accepts at most one embedded sync-wait per
    instruction; peel extras onto wait-only NoOps on the same engine."""
    for func in nc.m.functions:
        for bb in func.blocks:
            insts = bb.instructions
            i = 0
            while i < len(insts):
                inst = insts[i]
                si = getattr(inst, "sync_info", None)
                if si is None or len(si.on_wait) <= max_waits:
                    i += 1
                    continue
                waits = list(si.on_wait)
                keep, extra = waits[:max_waits], waits[max_waits:]
                nops = []
                while extra:
                    chunk, extra = extra[:max_waits], extra[max_waits:]
                    nop = mybir.InstNoOp(
                        name=f"{inst.name}_ws{len(nops)}", ins=[], outs=[])
                    nop.engine = inst.engine
                    nop.sync_info = mybir.SyncInfo(on_wait=chunk, on_update=[])
                    nc.register_instruction(nop, overwrite=True)
                    nops.append(nop)
                si.on_wait = keep
                for j, nop in enumerate(nops):
                    insts.insert(i + j, nop)
                i += len(nops) + 1


def build_nc(repeat: int = 1):
    nc = bass.Bass()
    xT = nc.dram_tensor("xT", [E, T], F32, kind="ExternalInput")
    wq_r = nc.dram_tensor("wq_r", [NCH, 128, NCH, 128], F32, kind="ExternalInput")
    wk_r = nc.dram_tensor("wk_r", [NCH, 128, NCH, 128], F32, kind="ExternalInput")
    wvT = nc.dram_tensor("wvT", [E, E], F32, kind="ExternalInput")
    woT = nc.dram_tensor("woT", [E, E], F32, kind="ExternalInput")
    bq_pm = nc.dram_tensor("bq_pm", [128, NCH], F32, kind="ExternalInput")
    bk_pm = nc.dram_tensor("bk_pm", [128, NCH], F32, kind="ExternalInput")
    bo_bc = nc.dram_tensor("bo_bc", [128, E], F32, kind="ExternalInput")
    out = nc.dram_tensor("out", [T, E], F32, kind="ExternalOutput")

    tril = np.where(np.arange(128)[None, :] >= np.arange(128)[:, None],
                    0.0, NEG).astype(np.float32)
    maskc = nc.inline_tensor(tril, name="maskc")
    warmc = nc.inline_tensor(np.zeros((128, 128), np.float32), name="warmc")

    with tile.TileContext(nc) as tc:
        from contextlib import ExitStack
        with ExitStack() as ctx:
            consts = ctx.enter_context(tc.tile_pool(name="consts", bufs=1))
            persist = ctx.enter_context(tc.tile_pool(name="persist", bufs=1))
            wqk_p = ctx.enter_context(tc.tile_pool(name="wqk", bufs=4))
            wrow_p = ctx.enter_context(tc.tile_pool(name="wrow", bufs=6))
            pt_p = ctx.enter_context(tc.tile_pool(name="pt", bufs=3))
            rb_p = ctx.enter_context(tc.tile_pool(name="rb", bufs=6))
            cu_p = ctx.enter_context(tc.tile_pool(name="cu", bufs=6))
            out_p = ctx.enter_context(tc.tile_pool(name="outp", bufs=4))
            drp = ctx.enter_context(tc.tile_pool(name="drp", bufs=2, space="DRAM"))

            def body():
                # PSUM pools are body-local: attention's pools (stp/ctxp/pp)
                # release before the tail so the final out-projection gets a
                # deep dedicated ring in the freed banks.
                psctx = ExitStack()
                pp = psctx.enter_context(
                    tc.tile_pool(name="pp", bufs=2, space="PSUM"))
                stp = psctx.enter_context(
                    tc.tile_pool(name="stp", bufs=2, space="PSUM"))
                ctxp = psctx.enter_context(
                    tc.tile_pool(name="ctxp", bufs=2, space="PSUM"))
                mask_sb = consts.tile([128, 128], F32)
                warm_sb = consts.tile([128, 128], F32R)
                bqs = consts.tile([128, NCH], F32)
                bks = consts.tile([128, NCH], F32)
                bos = consts.tile([128, E], F32)

                xt_sb = persist.tile([128, NCH, T], F32R)
                qt_sb = persist.tile([128, NCH, T], F32R)
                kt_sb = persist.tile([128, NCH, T], F32R)
                v65_sb = persist.tile([128, NTC, H * 65], F32R)
                ctxT_sb = persist.tile([128, NCH, T], F32R)
                # reciprocal staging: dens at partitions 0 / 32 (quadrant-
                # aligned single-partition DVE writes), double-buffered
                rt_all = persist.tile([33, 2, 512], F32)

                # --- input / const DMAs ---
                # first-needed first, alternating the sync/scalar HWDGE rings
                # so x chunks land every ~0.8us instead of serializing on one
                # FIFO ring.
                nc.vector.memset(warm_sb.bitcast(F32), 0.0)
                nc.sync.dma_start(out=xt_sb[:, 0, :],
                                  in_=xT[0:128, :].bitcast(F32R))
                nc.scalar.dma_start(out=xt_sb[:, 1, :],
                                    in_=xT[128:256, :].bitcast(F32R))
                wq_t0 = wqk_p.tile([128, NCH, 128], F32R, tag="wqk")
                nc.sync.dma_start(out=wq_t0, in_=wq_r[0, :, :, :].bitcast(F32R))
                wk_t0 = wqk_p.tile([128, NCH, 128], F32R, tag="wqk")
                nc.scalar.dma_start(out=wk_t0, in_=wk_r[0, :, :, :].bitcast(F32R))
                for ch in range(2, NCH):
                    eng = nc.sync if ch % 2 == 0 else nc.scalar
                    eng.dma_start(out=xt_sb[:, ch, :],
                                  in_=xT[ch * 128:(ch + 1) * 128, :].bitcast(F32R))
                nc.gpsimd.dma_start(out=bqs, in_=bq_pm[:, :])
                nc.gpsimd.dma_start(out=bks, in_=bk_pm[:, :])
                nc.gpsimd.dma_start(out=mask_sb, in_=maskc[:, :])
                nc.gpsimd.dma_start(out=bos, in_=bo_bc[:, :])
                wv_t = {}
                engs = [nc.sync, nc.scalar, nc.gpsimd]
                for ch in range(NCH):
                    w = wrow_p.tile([128, E], F32R, tag="wrow")
                    engs[ch % 3].dma_start(
                        out=w, in_=wvT[ch * 128:(ch + 1) * 128, :].bitcast(F32R))
                    wv_t[ch] = w

                # PE warmup while input DMAs stream: ride out the cold-clock
                # window on throwaway matmuls so real work starts warm
                wps = pp.tile([128, 128], F32, tag="pp", name="wps")
                for _ in range(14):
                    nc.tensor.matmul(wps, warm_sb, warm_sb,
                                     start=True, stop=True)
                # prime the ACT exp table during the DMA phase so the first
                # real exp doesn't pay the table-load latency
                expw = consts.tile([1, 8], F32)
                nc.scalar.activation(expw, warm_sb.bitcast(F32)[0:1, 0:8],
                                     mybir.ActivationFunctionType.Exp)

                # ones columns of V65 (col 64 of every head), one strided memset
                v65_r = v65_sb.rearrange("p k (h e) -> p k h e", e=65)
                nc.vector.memset(v65_r[:, :, :, 64:65].bitcast(F32), 1.0)

                # --- PE work-item generators (projections as filler) ---
                def proj_qk_win(m, w_t, bias_sb, dst_sb, copy_eng, win):
                    """Yield thunks: 6 accumulating matmuls + 1 copy."""
                    ps = pp.tile([128, 512], F32, tag="pp", name=f"qk{m}w{win}")
                    for ch in range(NCH):
                        def mm(ps=ps, ch=ch, win=win):
                            nc.tensor.matmul(
                                ps, w_t[:, ch, :],
                                xt_sb[:, ch, win * 512:(win + 1) * 512],
                                start=(ch == 0), stop=(ch == NCH - 1))
                        yield mm
                    def cp(ps=ps, win=win):
                        dst = dst_sb[:, m, win * 512:(win + 1) * 512]
                        if copy_eng == "act":
                            nc.scalar.activation(
                                dst, ps,
                                mybir.ActivationFunctionType.Identity,
                                bias=bias_sb[:, m:m + 1])
                        else:
                            nc.vector.tensor_scalar_add(
                                dst, ps, bias_sb[:, m:m + 1])
                    yield cp

                def proj_qk_items(m, wq, wk):
                    """Q/K projections for pair m, ordered so window-0 Q and K
                    are both ready as early as possible."""
                    yield from proj_qk_win(m, wq, bqs, qt_sb, "dve", 0)
                    yield from proj_qk_win(m, wk, bks, kt_sb, "dve", 0)
                    yield from proj_qk_win(m, wq, bqs, qt_sb, "dve", 1)
                    yield from proj_qk_win(m, wk, bks, kt_sb, "dve", 1)

                def proj_v_items(kc):
                    ps0 = pp.tile([128, 512], F32, tag="pp")
                    ps1 = pp.tile([128, 256], F32, tag="pp")
                    for ch in range(NCH):
                        def mm(ps0=ps0, ps1=ps1, ch=ch, kc=kc):
                            lhsT = xt_sb[:, ch, kc * 128:(kc + 1) * 128]
                            nc.tensor.matmul(ps0, lhsT, wv_t[ch][:, 0:512],
                                             start=(ch == 0), stop=(ch == NCH - 1))
                            nc.tensor.matmul(ps1, lhsT, wv_t[ch][:, 512:768],
                                             start=(ch == 0), stop=(ch == NCH - 1))
                        yield mm
                    def cp(ps0=ps0, ps1=ps1, kc=kc):
                        v65r = v65_sb[:, kc, :].rearrange("p (h e) -> p h e", e=65)
                        nc.scalar.copy(v65r[:, 0:8, 0:64], ps0)
                        nc.scalar.copy(v65r[:, 8:12, 0:64], ps1)
                    yield cp

                def emit(it, n):
                    """Run up to n filler items; return #emitted."""
                    k = 0
                    for f in it:
                        f()
                        k += 1
                        if k >= n:
                            break
                    return k

                def attn_win(p, win, filler=iter(()), fpk=1):
                    """Attention for head pair p (heads 2p, 2p+1), q window win.
                    filler: iterator of PE work thunks interleaved per k-chunk."""
                    nk = 4 * (win + 1)
                    w0 = win * 512
                    hA, hB = 2 * p, 2 * p + 1
                    ctxA = ctxp.tile([65, 512], F32, tag="ctx")
                    ctxB = ctxp.tile([65, 512], F32, tag="ctx")
                    pend = None  # (kc, pt, off) awaiting PV
                    for kc in range(nk):
                        off = max(kc * 128 - w0, 0)
                        st = stp.tile([128, 1024], F32, tag="st")
                        diag = kc * 128 - w0 >= 0
                        soff = 256 if off == 384 else off
                        nc.tensor.matmul(
                            st[:, soff:512],
                            kt_sb[0:64, p, kc * 128:(kc + 1) * 128],
                            qt_sb[0:64, p, w0 + soff:w0 + 512],
                            start=True, stop=True, tile_position=(0, 0))
                        nc.tensor.matmul(
                            st[:, 512 + soff:1024],
                            kt_sb[64:128, p, kc * 128:(kc + 1) * 128],
                            qt_sb[64:128, p, w0 + soff:w0 + 512],
                            start=True, stop=True, tile_position=(64, 0))
                        if diag:
                            # one strided op masks both heads' diagonal
                            # blocks; the mask operand broadcasts via a
                            # free-dim stride-0 AP
                            st4 = st.rearrange("p (s q) -> p s q", s=2)
                            view = st4[:, :, off:off + 128]
                            m2 = mask_sb[:, :]
                            mask_b = bass.AP(
                                tensor=m2.tensor, offset=m2.offset,
                                ap=[list(m2.ap)[0], [0, 2], list(m2.ap)[1]])
                            nc.vector.tensor_tensor(
                                out=view, in0=view, in1=mask_b,
                                op=mybir.AluOpType.add)
                        # exp -> bf16 P^T (ACT)
                        pt = pt_p.tile([128, 1024], F32R, tag="pt")
                        if off > 0:
                            st3 = st.rearrange("p (s q) -> p s q", s=2)
                            pt3 = pt.rearrange("p (s q) -> p s q", s=2)
                            nc.scalar.activation(
                                pt3[:, :, off:512], st3[:, :, off:512],
                                mybir.ActivationFunctionType.Exp)
                        else:
                            nc.scalar.activation(
                                pt, st, mybir.ActivationFunctionType.Exp)
                        emit(filler, fpk)
                        if pend is not None:
                            pkc, ppt, poff = pend
                            nc.tensor.matmul(
                                ctxA[:, poff:512],
                                v65_sb[:, pkc, hA * 65:hA * 65 + 65],
                                ppt[:, poff:512],
                                start=(pkc == 0), stop=False)
                            nc.tensor.matmul(
                                ctxB[:, poff:512],
                                v65_sb[:, pkc, hB * 65:hB * 65 + 65],
                                ppt[:, 512 + poff:1024],
                                start=(pkc == 0), stop=False)
                        pend = (kc, pt, off)
                    # drain last PV
                    pkc, ppt, poff = pend
                    nc.tensor.matmul(
                        ctxA[:, poff:512],
                        v65_sb[:, pkc, hA * 65:hA * 65 + 65],
                        ppt[:, poff:512],
                        start=(pkc == 0), stop=True)
                    nc.tensor.matmul(
                        ctxB[:, poff:512],
                        v65_sb[:, pkc, hB * 65:hB * 65 + 65],
                        ppt[:, 512 + poff:1024],
                        start=(pkc == 0), stop=True)
                    emit(filler, 2)
                    # Normalize: r = 1/den (DVE, straight from the PSUM den
                    # row), broadcast r across 64 partitions with GPSIMD's
                    # native partition_broadcast (no DRAM bounce), then
                    # head A multiplies PSUM ctx directly on DVE while head B
                    # goes ACT-copy -> GPSIMD multiply so the two heads'
                    # normalize work lands on different engines.
                    # Copy each head's ctx+den PSUM tile to SBUF in ONE op
                    # (cuA on DVE, cuB on ACT) so the PSUM ctx slots free
                    # after a single instruction; reciprocals then read the
                    # SBUF copies, off the ctx-recycle path.
                    cuA = cu_p.tile([65, 512], F32, tag="cu", name="cuA")
                    nc.vector.tensor_copy(cuA, ctxA)
                    cuB = cu_p.tile([65, 512], F32, tag="cu", name="cuB")
                    nc.scalar.copy(cuB, ctxB)
                    rt = rt_all[:, (2 * p + win) % 2, :]
                    nc.vector.reciprocal(rt[0:1, :], cuA[64:65, :])
                    nc.vector.reciprocal(rt[32:33, :], cuB[64:65, :])
                    sc = drp.tile([2, 512], F32, tag="dr", name="sc")
                    nc.sync.dma_start(out=sc[0:1, :], in_=rt[0:1, :])
                    nc.sync.dma_start(out=sc[1:2, :], in_=rt[32:33, :])
                    rbA = rb_p.tile([64, 512], F32, tag="rb", name="rbA")
                    nc.gpsimd.dma_start(
                        out=rbA,
                        in_=bass.AP(tensor=sc.tensor, offset=sc[0:1, :].offset,
                                    ap=[[0, 64], [1, 512]]))
                    rbB = rb_p.tile([64, 512], F32, tag="rb", name="rbB")
                    nc.gpsimd.dma_start(
                        out=rbB,
                        in_=bass.AP(tensor=sc.tensor, offset=sc[1:2, :].offset,
                                    ap=[[0, 64], [1, 512]]))
                    nc.gpsimd.tensor_tensor(
                        out=ctxT_sb[0:64, p, w0:w0 + 512],
                        in0=cuA[0:64, :], in1=rbA[:, :],
                        op=mybir.AluOpType.mult)
                    nc.gpsimd.tensor_tensor(
                        out=ctxT_sb[64:128, p, w0:w0 + 512],
                        in0=cuB[0:64, :], in1=rbB[:, :],
                        op=mybir.AluOpType.mult)


                def outproj_items(tcns, pool0=None, tag0="pp", tag1="pp"):
                    for tcn in tcns:
                        pl = pool0 if pool0 is not None else pp
                        ps0 = pl.tile([128, 512], F32, tag=tag0,
                                      name=f"ops0_{tcn}")
                        ps1 = pl.tile([128, 256], F32, tag=tag1,
                                      name=f"ops1_{tcn}")
                        for ch in range(NCH):
                            def mm(ps0=ps0, ps1=ps1, ch=ch, tcn=tcn):
                                lhsT = ctxT_sb[:, ch, tcn * 128:(tcn + 1) * 128]
                                nc.tensor.matmul(
                                    ps0, lhsT, wo_t[ch][:, 0:512],
                                    start=(ch == 0), stop=(ch == NCH - 1))
                                nc.tensor.matmul(
                                    ps1, lhsT, wo_t[ch][:, 512:768],
                                    start=(ch == 0), stop=(ch == NCH - 1))
                            yield mm
                        def fin(ps0=ps0, ps1=ps1, tcn=tcn):
                            # 3x256-wide bias+store pieces on three DMA
                            # queues so the final drain is one short piece
                            ot = out_p.tile([128, E], F32, tag="outp")
                            nc.vector.tensor_tensor(
                                out=ot[:, 0:256], in0=ps0[:, 0:256],
                                in1=bos[:, 0:256], op=mybir.AluOpType.add)
                            nc.sync.dma_start(
                                out=out[tcn * 128:(tcn + 1) * 128, 0:256],
                                in_=ot[:, 0:256])
                            nc.vector.tensor_tensor(
                                out=ot[:, 256:512], in0=ps0[:, 256:512],
                                in1=bos[:, 256:512], op=mybir.AluOpType.add)
                            nc.scalar.dma_start(
                                out=out[tcn * 128:(tcn + 1) * 128, 256:512],
                                in_=ot[:, 256:512])
                            nc.vector.tensor_tensor(
                                out=ot[:, 512:768], in0=ps1,
                                in1=bos[:, 512:768], op=mybir.AluOpType.add)
                            nc.gpsimd.dma_start(
                                out=out[tcn * 128:(tcn + 1) * 128, 512:768],
                                in_=ot[:, 512:768])
                        yield fin

                # --- emission schedule ---
                # Pair 0 Q/K projections up front (no filler context yet).
                for f in proj_qk_items(0, wq_t0, wk_t0):
                    f()
                # prefetch pair-1 weights
                wq_t1 = wqk_p.tile([128, NCH, 128], F32R, tag="wqk")
                nc.sync.dma_start(out=wq_t1, in_=wq_r[1, :, :, :].bitcast(F32R))
                wk_t1 = wqk_p.tile([128, NCH, 128], F32R, tag="wqk")
                nc.sync.dma_start(out=wk_t1, in_=wk_r[1, :, :, :].bitcast(F32R))
                # V projection for k-chunks 0..3 (needed by win 0)
                for kc in range(4):
                    for f in proj_v_items(kc):
                        f()

                def chain(*gens):
                    for g in gens:
                        yield from g

                # attn(0, win0) with V k-chunks 4..7 as filler (2 items/kc)
                fill = chain(*[proj_v_items(kc) for kc in range(4, NTC)])
                attn_win(0, 0, fill, fpk=2)
                emit(fill, 100)  # drain any leftover V items
                # attn(0, win1) with pair-1 Q/K proj as filler
                fill = proj_qk_items(1, wq_t1, wk_t1)
                attn_win(0, 1, fill, fpk=2)
                emit(fill, 100)

                wq_t = {0: wq_t0, 1: wq_t1}
                wk_t = {0: wk_t0, 1: wk_t1}
                wo_t = {}
                for p in range(1, NCH):
                    # prefetch next pair's weights / wo rows
                    if p + 1 < NCH:
                        wq_t[p + 1] = wqk_p.tile([128, NCH, 128], F32R, tag="wqk", name=f"wq_t{p+1}")
                        nc.sync.dma_start(out=wq_t[p + 1],
                                          in_=wq_r[p + 1, :, :, :].bitcast(F32R))
                        wk_t[p + 1] = wqk_p.tile([128, NCH, 128], F32R, tag="wqk", name=f"wk_t{p+1}")
                        nc.sync.dma_start(out=wk_t[p + 1],
                                          in_=wk_r[p + 1, :, :, :].bitcast(F32R))
                    else:
                        for ch in range(NCH):
                            w = wrow_p.tile([128, E], F32R, tag="wrow")
                            nc.sync.dma_start(
                                out=w,
                                in_=woT[ch * 128:(ch + 1) * 128, :].bitcast(F32R))
                            wo_t[ch] = w
                    if p + 1 < NCH:
                        fill = proj_qk_items(p + 1, wq_t[p + 1], wk_t[p + 1])
                        attn_win(p, 0, fill, fpk=2)
                        attn_win(p, 1, fill, fpk=2)
                        emit(fill, 100)
                    else:
                        # last pair: window-1 PE gaps are filled with the
                        # t-chunks of the output projection that only need
                        # this pair's window-0 context (t < 512)
                        attn_win(p, 0, iter(()))
                        fill = outproj_items(range(4))
                        attn_win(p, 1, fill, fpk=2)
                        emit(fill, 100)

                # --- output projection, second half ---
                # attention's PSUM pools release here; the tail runs out of a
                # dedicated 4-deep ring so all four t-chunks pipeline.
                psctx.close()
                with tc.tile_pool(name="po", bufs=4, space="PSUM") as po:
                    for f in outproj_items(range(4, NTC - 1), po, "po0", "po1"):
                        f()
                    # last t-chunk column-major: each 256-wide third finishes
                    # its accumulation, bias-adds and stores while the next
                    # third's matmuls still stream, shortening the drain
                    tcn = NTC - 1
                    ps0 = po.tile([128, 512], F32, tag="po0")
                    ps1 = po.tile([128, 256], F32, tag="po1")
                    ot = out_p.tile([128, E], F32, tag="outp")
                    pieces = [(0, ps0[:, 0:256], nc.sync),
                              (256, ps0[:, 256:512], nc.scalar),
                              (512, ps1[:, :], nc.gpsimd)]
                    for c0, psl, eng in pieces:
                        for ch in range(NCH):
                            nc.tensor.matmul(
                                psl, ctxT_sb[:, ch, tcn * 128:(tcn + 1) * 128],
                                wo_t[ch][:, c0:c0 + 256],
                                start=(ch == 0), stop=(ch == NCH - 1))
                        nc.vector.tensor_tensor(
                            out=ot[:, c0:c0 + 256], in0=psl,
                            in1=bos[:, c0:c0 + 256], op=mybir.AluOpType.add)
                        eng.dma_start(
                            out=out[tcn * 128:(tcn + 1) * 128, c0:c0 + 256],
                            in_=ot[:, c0:c0 + 256])

            for _rep in range(repeat):
                body()

    _split_excess_waits(nc)
    return nc


_NC_CACHE = None


def _make_in_maps(x, Wq, bq, Wk, bk, Wv, bv, Wo, bo):
    scale = 1.0 / np.sqrt(D)

    def reorder(w):
        return np.ascontiguousarray(
            w.T.reshape(NCH, 128, NCH, 128).transpose(2, 1, 0, 3))

    wq_r = reorder((Wq.astype(np.float64) * scale).astype(np.float32))
    wk_r = reorder(Wk.astype(np.float32))
    wvT = np.ascontiguousarray(Wv.T.astype(np.float32))
    woT = np.ascontiguousarray(Wo.T.astype(np.float32))
    bq_pm = np.ascontiguousarray(
        (bq.astype(np.float64) * scale).astype(np.float32).reshape(NCH, 128).T)
    bk_pm = np.ascontiguousarray(bk.reshape(NCH, 128).T.astype(np.float32))
    bo_eff = (bo.astype(np.float64)
              + bv.astype(np.float64) @ Wo.T.astype(np.float64)).astype(np.float32)
    bo_bc = np.ascontiguousarray(np.tile(bo_eff[None, :], (128, 1)))
    maps = []
    for b in range(B):
        xTb = np.ascontiguousarray(x[b].T.astype(np.float32))
        maps.append({"xT": xTb, "wq_r": wq_r, "wk_r": wk_r, "wvT": wvT,
                     "woT": woT, "bq_pm": bq_pm, "bk_pm": bk_pm, "bo_bc": bo_bc})
    return maps


def kernel(x, Wq, bq, Wk, bk, Wv, bv, Wo, bo):
    global _NC_CACHE
    from concourse.bass_utils import run_bass_kernel_spmd
    if _NC_CACHE is None:
        _NC_CACHE = build_nc()
    in_maps = _make_in_maps(x, Wq, bq, Wk, bk, Wv, bv, Wo, bo)
    res = run_bass_kernel_spmd(_NC_CACHE, in_maps, core_ids=list(range(B)))
    return np.stack([res.results[i]["out"] for i in range(B)], axis=0)



# revision 25
# speedup vs baseline: 2.4680x; 2.0601x over previous
"""Causal multi-head attention (B=8, T=1024, E=768, H=12, D=64) on 8 trn2
NeuronCores, data-parallel over the batch (one batch element per core).

v3 — f32r matmul datapath (bf16 emits LDWEIGHTS-per-matmul which serializes
on this walrus build) with fine-grained projection/attention interleaving.

Per-core pipeline (all matmuls float32r — full PE rate, ~1e-4 rel err):
  1. PE warmup matmuls ride out the cold-clock window while x/W DMAs stream
     (first-needed-first DMA order across the sync + scalar DGE queues).
  2. Q^T = (Wq/8) @ x^T + bq/8, K^T = Wk @ x^T + bk -> SBUF f32r [768, 1024]
     (softmax 1/sqrt(D) scale folded into Wq/bq on the host).
  3. V = x @ Wv^T (bias folded into the output projection bias) -> SBUF
     [1024, 12*65] with a ones column per head ("V65") so the attention
     context matmul also produces the softmax denominator.
  4. Per head pair, per 512-wide q window: S^T[k,q] via row-packed matmuls
     (K=64, tile_position (0,0)/(64,0)); the 128-wide triangle-edge tiles
     are widened to 256 into the never-read masked zone to dodge the f32r
     sub-256 4-cycle/row penalty; causal mask added on both heads'
     diagonal 128-blocks by one strided DVE op (free-dim stride-0 mask
     broadcast), exp on ACT -> f32r P^T, ctx^T[65,q] accumulated over
     k-chunks with V65 stationary.  Projection matmuls for the NEXT head
     pair are emitted between the k-chunk steps as PE filler so exp latency
     never stalls the in-order PE queue.
  5. Normalization: ctx is copied PSUM->SBUF on ACT (its window-boundary
     lull) right after the last PV so the PSUM accumulators free for the
     next window without queuing behind DVE; then (off the critical path) reciprocal of row 64 on DVE (partitions 0/32), broadcast across
     partitions via a DRAM-bounce DMA (stride-0 partition source; a PE
     rank-2 broadcast matmul measures ~32us on this HW), and GPSIMD
     multiplies (all-SBUF operands, off the loaded DVE) -> ctxT f32r.
  6. out = ctx_norm @ Wo^T + bo_eff (bo_eff = bo + bv @ Wo^T).  The t<512
     half runs interleaved into the last pair's window-1 attention (its
     context is complete after window 0); bias add and output DMA are split
     512/256 across the sync/scalar queues to shorten the tail.
"""
import sys
import numpy as np

sys.path.insert(0, "/opt/trn_rl_repo")

import concourse.bass as bass
import concourse.mybir as mybir
import concourse.tile as tile

F32 = mybir.dt.float32
F32R = mybir.dt.float32r
BF16 = mybir.dt.bfloat16

B, T, E, H, D = 8, 1024, 768, 12, 64
NCH = E // 128          # 6 e-chunks
NTC = T // 128          # 8 t-chunks
NW = T // 512           # 2 q-windows
NEG = -1.0e9


def _split_excess_waits(nc, max_waits: int = 1):
    """walrus on this stack accepts at most one embedded sync-wait per
    instruction; peel extras onto wait-only NoOps on the same engine."""
    for func in nc.m.functions:
        for bb in func.blocks:
            insts = bb.instructions
            i = 0
            while i < len(insts):
                inst = insts[i]
                si = getattr(inst, "sync_info", None)
                if si is None or len(si.on_wait) <= max_waits:
                    i += 1
                    continue
                waits = list(si.on_wait)
                keep, extra = waits[:max_waits], waits[max_waits:]
                nops = []
                while extra:
                    chunk, extra = extra[:max_waits], extra[max_waits:]
                    nop = mybir.InstNoOp(
                        name=f"{inst.name}_ws{len(nops)}", ins=[], outs=[])
                    nop.engine = inst.engine
                    nop.sync_info = mybir.SyncInfo(on_wait=chunk, on_update=[])
                    nc.register_instruction(nop, overwrite=True)
                    nops.append(nop)
                si.on_wait = keep
                for j, nop in enumerate(nops):
                    insts.insert(i + j, nop)
                i += len(nops) + 1


def build_nc(repeat: int = 1):
    nc = bass.Bass()
    xT = nc.dram_tensor("xT", [E, T], F32, kind="ExternalInput")
    wq_r = nc.dram_tensor("wq_r", [NCH, 128, NCH, 128], F32, kind="ExternalInput")
    wk_r = nc.dram_tensor("wk_r", [NCH, 128, NCH, 128], F32, kind="ExternalInput")
    wvT = nc.dram_tensor("wvT", [E, E], F32, kind="ExternalInput")
    woT = nc.dram_tensor("woT", [E, E], F32, kind="ExternalInput")
    bq_pm = nc.dram_tensor("bq_pm", [128, NCH], F32, kind="ExternalInput")
    bk_pm = nc.dram_tensor("bk_pm", [128, NCH], F32, kind="ExternalInput")
    bo_bc = nc.dram_tensor("bo_bc", [128, E], F32, kind="ExternalInput")
    out = nc.dram_tensor("out", [T, E], F32, kind="ExternalOutput")

    tril = np.where(np.arange(128)[None, :] >= np.arange(128)[:, None],
                    0.0, NEG).astype(np.float32)
    maskc = nc.inline_tensor(tril, name="maskc")
    warmc = nc.inline_tensor(np.zeros((128, 128), np.float32), name="warmc")

    with tile.TileContext(nc) as tc:
        from contextlib import ExitStack
        with ExitStack() as ctx:
            consts = ctx.enter_context(tc.tile_pool(name="consts", bufs=1))
            persist = ctx.enter_context(tc.tile_pool(name="persist", bufs=1))
            wqk_p = ctx.enter_context(tc.tile_pool(name="wqk", bufs=4))
            wrow_p = ctx.enter_context(tc.tile_pool(name="wrow", bufs=6))
            pt_p = ctx.enter_context(tc.tile_pool(name="pt", bufs=3))
            rb_p = ctx.enter_context(tc.tile_pool(name="rb", bufs=6))
            cu_p = ctx.enter_context(tc.tile_pool(name="cu", bufs=6))
            out_p = ctx.enter_context(tc.tile_pool(name="outp", bufs=4))
            drp = ctx.enter_context(tc.tile_pool(name="drp", bufs=2, space="DRAM"))

            def body():
                # PSUM pools are body-local: attention's pools (stp/ctxp/pp)
                # release before the tail so the final out-projection gets a
                # deep dedicated ring in the freed banks.
                psctx = ExitStack()
                pp = psctx.enter_context(
                    tc.tile_pool(name="pp", bufs=2, space="PSUM"))
                stp = psctx.enter_context(
                    tc.tile_pool(name="stp", bufs=2, space="PSUM"))
                ctxp = psctx.enter_context(
                    tc.tile_pool(name="ctxp", bufs=2, space="PSUM"))
                mask_sb = consts.tile([128, 128], F32)
                warm_sb = consts.tile([128, 128], F32R)
                bqs = consts.tile([128, NCH], F32)
                bks = consts.tile([128, NCH], F32)
                bos = consts.tile([128, E], F32)

                xt_sb = persist.tile([128, NCH, T], F32R)
                qt_sb = persist.tile([128, NCH, T], F32R)
                kt_sb = persist.tile([128, NCH, T], F32R)
                v65_sb = persist.tile([128, NTC, H * 65], F32R)
                ctxT_sb = persist.tile([128, NCH, T], F32R)
                # reciprocal staging: dens at partitions 0 / 32 (quadrant-
                # aligned single-partition DVE writes), double-buffered
                rt_all = persist.tile([33, 2, 512], F32)

                # --- input / const DMAs ---
                # first-needed first, alternating the sync/scalar HWDGE rings
                # so x chunks land every ~0.8us instead of serializing on one
                # FIFO ring.
                nc.vector.memset(warm_sb.bitcast(F32), 0.0)
                nc.sync.dma_start(out=xt_sb[:, 0, :],
                                  in_=xT[0:128, :].bitcast(F32R))
                nc.scalar.dma_start(out=xt_sb[:, 1, :],
                                    in_=xT[128:256, :].bitcast(F32R))
                wq_t0 = wqk_p.tile([128, NCH, 128], F32R, tag="wqk")
                nc.sync.dma_start(out=wq_t0, in_=wq_r[0, :, :, :].bitcast(F32R))
                wk_t0 = wqk_p.tile([128, NCH, 128], F32R, tag="wqk")
                nc.scalar.dma_start(out=wk_t0, in_=wk_r[0, :, :, :].bitcast(F32R))
                for ch in range(2, NCH):
                    eng = nc.sync if ch % 2 == 0 else nc.scalar
                    eng.dma_start(out=xt_sb[:, ch, :],
                                  in_=xT[ch * 128:(ch + 1) * 128, :].bitcast(F32R))
                nc.gpsimd.dma_start(out=bqs, in_=bq_pm[:, :])
                nc.gpsimd.dma_start(out=bks, in_=bk_pm[:, :])
                nc.gpsimd.dma_start(out=mask_sb, in_=maskc[:, :])
                nc.gpsimd.dma_start(out=bos, in_=bo_bc[:, :])
                wv_t = {}
                engs = [nc.sync, nc.scalar, nc.gpsimd]
                for ch in range(NCH):
                    w = wrow_p.tile([128, E], F32R, tag="wrow")
                    engs[ch % 3].dma_start(
                        out=w, in_=wvT[ch * 128:(ch + 1) * 128, :].bitcast(F32R))
                    wv_t[ch] = w

                # PE warmup while input DMAs stream: ride out the cold-clock
                # window on throwaway matmuls so real work starts warm
                wps = pp.tile([128, 128], F32, tag="pp", name="wps")
                for _ in range(14):
                    nc.tensor.matmul(wps, warm_sb, warm_sb,
                                     start=True, stop=True)
                # prime the ACT exp table during the DMA phase so the first
                # real exp doesn't pay the table-load latency
                expw = consts.tile([1, 8], F32)
                nc.scalar.activation(expw, warm_sb.bitcast(F32)[0:1, 0:8],
                                     mybir.ActivationFunctionType.Exp)

                # ones columns of V65 (col 64 of every head), one strided memset
                v65_r = v65_sb.rearrange("p k (h e) -> p k h e", e=65)
                nc.vector.memset(v65_r[:, :, :, 64:65].bitcast(F32), 1.0)

                # --- PE work-item generators (projections as filler) ---
                def proj_qk_win(m, w_t, bias_sb, dst_sb, copy_eng, win):
                    """Yield thunks: 6 accumulating matmuls + 1 copy."""
                    ps = pp.tile([128, 512], F32, tag="pp", name=f"qk{m}w{win}")
                    for ch in range(NCH):
                        def mm(ps=ps, ch=ch, win=win):
                            nc.tensor.matmul(
                                ps, w_t[:, ch, :],
                                xt_sb[:, ch, win * 512:(win + 1) * 512],
                                start=(ch == 0), stop=(ch == NCH - 1))
                        yield mm
                    def cp(ps=ps, win=win):
                        dst = dst_sb[:, m, win * 512:(win + 1) * 512]
                        if copy_eng == "act":
                            nc.scalar.activation(
                                dst, ps,
                                mybir.ActivationFunctionType.Identity,
                                bias=bias_sb[:, m:m + 1])
                        else:
                            nc.vector.tensor_scalar_add(
                                dst, ps, bias_sb[:, m:m + 1])
                    yield cp

                def proj_qk_items(m, wq, wk):
                    """Q/K projections for pair m, ordered so window-0 Q and K
                    are both ready as early as possible."""
                    yield from proj_qk_win(m, wq, bqs, qt_sb, "dve", 0)
                    yield from proj_qk_win(m, wk, bks, kt_sb, "dve", 0)
                    yield from proj_qk_win(m, wq, bqs, qt_sb, "dve", 1)
                    yield from proj_qk_win(m, wk, bks, kt_sb, "dve", 1)

                def proj_v_items(kc):
                    ps0 = pp.tile([128, 512], F32, tag="pp")
                    ps1 = pp.tile([128, 256], F32, tag="pp")
                    for ch in range(NCH):
                        def mm(ps0=ps0, ps1=ps1, ch=ch, kc=kc):
                            lhsT = xt_sb[:, ch, kc * 128:(kc + 1) * 128]
                            nc.tensor.matmul(ps0, lhsT, wv_t[ch][:, 0:512],
                                             start=(ch == 0), stop=(ch == NCH - 1))
                            nc.tensor.matmul(ps1, lhsT, wv_t[ch][:, 512:768],
                                             start=(ch == 0), stop=(ch == NCH - 1))
                        yield mm
                    def cp(ps0=ps0, ps1=ps1, kc=kc):
                        v65r = v65_sb[:, kc, :].rearrange("p (h e) -> p h e", e=65)
                        nc.scalar.copy(v65r[:, 0:8, 0:64], ps0)
                        nc.scalar.copy(v65r[:, 8:12, 0:64], ps1)
                    yield cp

                def emit(it, n):
                    """Run up to n filler items; return #emitted."""
                    k = 0
                    for f in it:
                        f()
                        k += 1
                        if k >= n:
                            break
                    return k

                def attn_win(p, win, filler=iter(()), fpk=1):
                    """Attention for head pair p (heads 2p, 2p+1), q window win.
                    filler: iterator of PE work thunks interleaved per k-chunk."""
                    nk = 4 * (win + 1)
                    w0 = win * 512
                    hA, hB = 2 * p, 2 * p + 1
                    ctxA = ctxp.tile([65, 512], F32, tag="ctx")
                    ctxB = ctxp.tile([65, 512], F32, tag="ctx")
                    pend = None  # (kc, pt, off) awaiting PV
                    for kc in range(nk):
                        off = max(kc * 128 - w0, 0)
                        st = stp.tile([128, 1024], F32, tag="st")
                        diag = kc * 128 - w0 >= 0
                        soff = 256 if off == 384 else off
                        nc.tensor.matmul(
                            st[:, soff:512],
                            kt_sb[0:64, p, kc * 128:(kc + 1) * 128],
                            qt_sb[0:64, p, w0 + soff:w0 + 512],
                            start=True, stop=True, tile_position=(0, 0))
                        nc.tensor.matmul(
                            st[:, 512 + soff:1024],
                            kt_sb[64:128, p, kc * 128:(kc + 1) * 128],
                            qt_sb[64:128, p, w0 + soff:w0 + 512],
                            start=True, stop=True, tile_position=(64, 0))
                        if diag:
                            # one strided op masks both heads' diagonal
                            # blocks; the mask operand broadcasts via a
                            # free-dim stride-0 AP
                            st4 = st.rearrange("p (s q) -> p s q", s=2)
                            view = st4[:, :, off:off + 128]
                            m2 = mask_sb[:, :]
                            mask_b = bass.AP(
                                tensor=m2.tensor, offset=m2.offset,
                                ap=[list(m2.ap)[0], [0, 2], list(m2.ap)[1]])
                            nc.vector.tensor_tensor(
                                out=view, in0=view, in1=mask_b,
                                op=mybir.AluOpType.add)
                        # exp -> bf16 P^T (ACT)
                        pt = pt_p.tile([128, 1024], F32R, tag="pt")
                        if off > 0:
                            st3 = st.rearrange("p (s q) -> p s q", s=2)
                            pt3 = pt.rearrange("p (s q) -> p s q", s=2)
                            nc.scalar.activation(
                                pt3[:, :, off:512], st3[:, :, off:512],
                                mybir.ActivationFunctionType.Exp)
                        else:
                            nc.scalar.activation(
                                pt, st, mybir.ActivationFunctionType.Exp)
                        emit(filler, fpk)
                        if pend is not None:
                            pkc, ppt, poff = pend
                            nc.tensor.matmul(
                                ctxA[:, poff:512],
                                v65_sb[:, pkc, hA * 65:hA * 65 + 65],
                                ppt[:, poff:512],
                                start=(pkc == 0), stop=False)
                            nc.tensor.matmul(
                                ctxB[:, poff:512],
                                v65_sb[:, pkc, hB * 65:hB * 65 + 65],
                                ppt[:, 512 + poff:1024],
                                start=(pkc == 0), stop=False)
                        pend = (kc, pt, off)
                    # drain last PV
                    pkc, ppt, poff = pend
                    nc.tensor.matmul(
                        ctxA[:, poff:512],
                        v65_sb[:, pkc, hA * 65:hA * 65 + 65],
                        ppt[:, poff:512],
                        start=(pkc == 0), stop=True)
                    nc.tensor.matmul(
                        ctxB[:, poff:512],
                        v65_sb[:, pkc, hB * 65:hB * 65 + 65],
                        ppt[:, 512 + poff:1024],
                        start=(pkc == 0), stop=True)
                    # Normalize. Copy each head's ctx+den PSUM tile to SBUF
                    # in ONE op (cuA on DVE, cuB on ACT) so the PSUM ctx
                    # slots free after a single instruction; reciprocals then
                    # read the SBUF copies, off the ctx-recycle path. These
                    # are emitted BEFORE any filler so they sit at the front
                    # of the DVE/ACT queues at window close.
                    cuA = cu_p.tile([65, 512], F32, tag="cu", name="cuA")
                    nc.vector.tensor_copy(cuA, ctxA)
                    cuB = cu_p.tile([65, 512], F32, tag="cu", name="cuB")
                    nc.scalar.copy(cuB, ctxB)
                    rt = rt_all[:, (2 * p + win) % 2, :]
                    nc.vector.reciprocal(rt[0:1, :], cuA[64:65, :])
                    nc.vector.reciprocal(rt[32:33, :], cuB[64:65, :])
                    emit(filler, 2)
                    sc = drp.tile([2, 512], F32, tag="dr", name="sc")
                    nc.sync.dma_start(out=sc[0:1, :], in_=rt[0:1, :])
                    nc.sync.dma_start(out=sc[1:2, :], in_=rt[32:33, :])
                    rbA = rb_p.tile([64, 512], F32, tag="rb", name="rbA")
                    nc.gpsimd.dma_start(
                        out=rbA,
                        in_=bass.AP(tensor=sc.tensor, offset=sc[0:1, :].offset,
                                    ap=[[0, 64], [1, 512]]))
                    rbB = rb_p.tile([64, 512], F32, tag="rb", name="rbB")
                    nc.gpsimd.dma_start(
                        out=rbB,
                        in_=bass.AP(tensor=sc.tensor, offset=sc[1:2, :].offset,
                                    ap=[[0, 64], [1, 512]]))
                    nc.gpsimd.tensor_tensor(
                        out=ctxT_sb[0:64, p, w0:w0 + 512],
                        in0=cuA[0:64, :], in1=rbA[:, :],
                        op=mybir.AluOpType.mult)
                    nc.gpsimd.tensor_tensor(
                        out=ctxT_sb[64:128, p, w0:w0 + 512],
                        in0=cuB[0:64, :], in1=rbB[:, :],
                        op=mybir.AluOpType.mult)


                def outproj_items(tcns, pool0=None, tag0="pp", tag1="pp"):
                    for tcn in tcns:
                        pl = pool0 if pool0 is not None else pp
                        ps0 = pl.tile([128, 512], F32, tag=tag0,
                                      name=f"ops0_{tcn}")
                        ps1 = pl.tile([128, 256], F32, tag=tag1,
                                      name=f"ops1_{tcn}")
                        for ch in range(NCH):
                            def mm(ps0=ps0, ps1=ps1, ch=ch, tcn=tcn):
                                lhsT = ctxT_sb[:, ch, tcn * 128:(tcn + 1) * 128]
                                nc.tensor.matmul(
                                    ps0, lhsT, wo_t[ch][:, 0:512],
                                    start=(ch == 0), stop=(ch == NCH - 1))
                                nc.tensor.matmul(
                                    ps1, lhsT, wo_t[ch][:, 512:768],
                                    start=(ch == 0), stop=(ch == NCH - 1))
                            yield mm
                        def fin(ps0=ps0, ps1=ps1, tcn=tcn):
                            # 3x256-wide bias+store pieces on three DMA
                            # queues so the final drain is one short piece
                            ot = out_p.tile([128, E], F32, tag="outp")
                            nc.vector.tensor_tensor(
                                out=ot[:, 0:256], in0=ps0[:, 0:256],
                                in1=bos[:, 0:256], op=mybir.AluOpType.add)
                            nc.sync.dma_start(
                                out=out[tcn * 128:(tcn + 1) * 128, 0:256],
                                in_=ot[:, 0:256])
                            nc.vector.tensor_tensor(
                                out=ot[:, 256:512], in0=ps0[:, 256:512],
                                in1=bos[:, 256:512], op=mybir.AluOpType.add)
                            nc.scalar.dma_start(
                                out=out[tcn * 128:(tcn + 1) * 128, 256:512],
                                in_=ot[:, 256:512])
                            nc.vector.tensor_tensor(
                                out=ot[:, 512:768], in0=ps1,
                                in1=bos[:, 512:768], op=mybir.AluOpType.add)
                            nc.gpsimd.dma_start(
                                out=out[tcn * 128:(tcn + 1) * 128, 512:768],
                                in_=ot[:, 512:768])
                        yield fin

                # --- emission schedule ---
                # Pair 0 Q/K projections up front (no filler context yet).
                for f in proj_qk_items(0, wq_t0, wk_t0):
                    f()
                # prefetch pair-1 weights
                wq_t1 = wqk_p.tile([128, NCH, 128], F32R, tag="wqk")
                nc.sync.dma_start(out=wq_t1, in_=wq_r[1, :, :, :].bitcast(F32R))
                wk_t1 = wqk_p.tile([128, NCH, 128], F32R, tag="wqk")
                nc.sync.dma_start(out=wk_t1, in_=wk_r[1, :, :, :].bitcast(F32R))
                # V projection for k-chunks 0..3 (needed by win 0)
                for kc in range(4):
                    for f in proj_v_items(kc):
                        f()

                def chain(*gens):
                    for g in gens:
                        yield from g

                # attn(0, win0) with V k-chunks 4..7 as filler (2 items/kc)
                fill = chain(*[proj_v_items(kc) for kc in range(4, NTC)])
                attn_win(0, 0, fill, fpk=2)
                emit(fill, 100)  # drain any leftover V items
                # attn(0, win1) with pair-1 Q/K proj as filler
                fill = proj_qk_items(1, wq_t1, wk_t1)
                attn_win(0, 1, fill, fpk=2)
                emit(fill, 100)

                wq_t = {0: wq_t0, 1: wq_t1}
                wk_t = {0: wk_t0, 1: wk_t1}
                wo_t = {}
                for p in range(1, NCH):
                    # prefetch next pair's weights / wo rows
                    if p + 1 < NCH:
                        wq_t[p + 1] = wqk_p.tile([128, NCH, 128], F32R, tag="wqk", name=f"wq_t{p+1}")
                        nc.sync.dma_start(out=wq_t[p + 1],
                                          in_=wq_r[p + 1, :, :, :].bitcast(F32R))
                        wk_t[p + 1] = wqk_p.tile([128, NCH, 128], F32R, tag="wqk", name=f"wk_t{p+1}")
                        nc.sync.dma_start(out=wk_t[p + 1],
                                          in_=wk_r[p + 1, :, :, :].bitcast(F32R))
                    else:
                        for ch in range(NCH):
                            w = wrow_p.tile([128, E], F32R, tag="wrow")
                            nc.sync.dma_start(
                                out=w,
                                in_=woT[ch * 128:(ch + 1) * 128, :].bitcast(F32R))
                            wo_t[ch] = w
                    if p + 1 < NCH:
                        fill = proj_qk_items(p + 1, wq_t[p + 1], wk_t[p + 1])
                        attn_win(p, 0, fill, fpk=2)
                        attn_win(p, 1, fill, fpk=2)
                        emit(fill, 100)
                    else:
                        # last pair: window-1 PE gaps are filled with the
                        # t-chunks of the output projection that only need
                        # this pair's window-0 context (t < 512)
                        attn_win(p, 0, iter(()))
                        fill = outproj_items(range(4))
                        attn_win(p, 1, fill, fpk=2)
                        emit(fill, 100)

                # --- output projection, second half ---
                # attention's PSUM pools release here; the tail runs out of a
                # dedicated 4-deep ring so all four t-chunks pipeline.
                psctx.close()
                with tc.tile_pool(name="po", bufs=4, space="PSUM") as po:
                    for f in outproj_items(range(4, NTC - 1), po, "po0", "po1"):
                        f()
                    # last t-chunk column-major: each 256-wide third finishes
                    # its accumulation, bias-adds and stores while the next
                    # third's matmuls still stream, shortening the drain
                    tcn = NTC - 1
                    ps0 = po.tile([128, 512], F32, tag="po0")
                    ps1 = po.tile([128, 256], F32, tag="po1")
                    ot = out_p.tile([128, E], F32, tag="outp")
                    pieces = [(0, ps0[:, 0:256], nc.sync),
                              (256, ps0[:, 256:512], nc.scalar),
                              (512, ps1[:, :], nc.gpsimd)]
                    for c0, psl, eng in pieces:
                        for ch in range(NCH):
                            nc.tensor.matmul(
                                psl, ctxT_sb[:, ch, tcn * 128:(tcn + 1) * 128],
                                wo_t[ch][:, c0:c0 + 256],
                                start=(ch == 0), stop=(ch == NCH - 1))
                        nc.vector.tensor_tensor(
                            out=ot[:, c0:c0 + 256], in0=psl,
                            in1=bos[:, c0:c0 + 256], op=mybir.AluOpType.add)
                        eng.dma_start(
                            out=out[tcn * 128:(tcn + 1) * 128, c0:c0 + 256],
                            in_=ot[:, c0:c0 + 256])

            for _rep in range(repeat):
                body()

    _split_excess_waits(nc)
    return nc


_NC_CACHE = None


def _make_in_maps(x, Wq, bq, Wk, bk, Wv, bv, Wo, bo):
    scale = 1.0 / np.sqrt(D)

    def reorder(w):
        return np.ascontiguousarray(
            w.T.reshape(NCH, 128, NCH, 128).transpose(2, 1, 0, 3))

    wq_r = reorder((Wq.astype(np.float64) * scale).astype(np.float32))
    wk_r = reorder(Wk.astype(np.float32))
    wvT = np.ascontiguousarray(Wv.T.astype(np.float32))
    woT = np.ascontiguousarray(Wo.T.astype(np.float32))
    bq_pm = np.ascontiguousarray(
        (bq.astype(np.float64) * scale).astype(np.float32).reshape(NCH, 128).T)
    bk_pm = np.ascontiguousarray(bk.reshape(NCH, 128).T.astype(np.float32))
    bo_eff = (bo.astype(np.float64)
              + bv.astype(np.float64) @ Wo.T.astype(np.float64)).astype(np.float32)
    bo_bc = np.ascontiguousarray(np.tile(bo_eff[None, :], (128, 1)))
    maps = []
    for b in range(B):
        xTb = np.ascontiguousarray(x[b].T.astype(np.float32))
        maps.append({"xT": xTb, "wq_r": wq_r, "wk_r": wk_r, "wvT": wvT,
                     "woT": woT, "bq_pm": bq_pm, "bk_pm": bk_pm, "bo_bc": bo_bc})
    return maps


def kernel(x, Wq, bq, Wk, bk, Wv, bv, Wo, bo):
    global _NC_CACHE
    from concourse.bass_utils import run_bass_kernel_spmd
    if _NC_CACHE is None:
        _NC_CACHE = build_nc()
    in_maps = _make_in_maps(x, Wq, bq, Wk, bk, Wv, bv, Wo, bo)
    res = run_bass_kernel_spmd(_NC_CACHE, in_maps, core_ids=list(range(B)))
    return np.stack([res.results[i]["out"] for i in range(B)], axis=0)

